# revision 17
# baseline (speedup 1.0000x reference)
"""MLA (multi-head latent attention) Trainium2 kernel, 8-way tensor/data parallel.

Problem shapes (hardcoded): B=2, S=2048, HID=2048, H=16, KVH=4, DH=128, L=64.

Sharding: core c -> batch b = c//4, kv-group g = c%4.
Each core computes q-heads 4g..4g+3 and kv head g for its batch.

Stage A: q/k/v projections j-outer (4 seq-quarter PSUM accumulators per pass,
  so consecutive matmuls share the stationary weight chunk), RoPE on DVE with
  bf16 trig (q trig pre-scaled by 1/sqrt(DH) so attention exp needs no scale),
  j-major hsT DMA so the k pass streams behind the DMA; latent branch last.
Stage B: flat software pipeline over all 64 (head, sq, k2) steps:
  exp(t) | scores(t+1) | av(t-1) | pair-sum(t-1), with the softmax denominator
  reduced by a DVE/GPSIMD add tree to one [128,512] tile -> single ones-matmul,
  reciprocal on DVE (PSUM src, bf16 out), broadcast matmul in bf16, and the
  whole normalization of block b deferred into block b+1 so the PE never
  stalls.  AllGather per head; head 3 in two halves.
Stage C: o-projection consumes gathered heads in ARRIVAL order (j-outer),
  parking partial sums in SBUF f32; only head-3's matmuls trail the last
  AllGather; latent contribution (ReduceScattered earlier) folded in as a
  bf16 rank-1 update.
"""

import numpy as np
import ml_dtypes
from contextlib import ExitStack

import concourse.bass as bass
import concourse.bacc as bacc
import concourse.tile as tile
import concourse.mybir as mybir
from concourse.bass_utils import run_bass_kernel_spmd

BF16 = ml_dtypes.bfloat16
FP32 = mybir.dt.float32
BF16_DT = mybir.dt.bfloat16

B, S, HID = 2, 2048, 2048
H, KVH, DH, L = 16, 4, 128, 64
THETA = 10000.0
N_CORES = 8
GROUPS = [[0, 1, 2, 3], [4, 5, 6, 7]]
NJ = HID // 128          # 16 contraction chunks
NSQ = S // 512           # 4 seq blocks of 512
NSB = S // 128           # 16 seq blocks of 128
SCALE = 1.0 / float(np.sqrt(np.float32(DH)))
NBLK = 16                # 4 heads x 4 sq blocks
NT = NBLK * 8            # flat pipeline steps (one per k2 chunk-pair)

_COMPILED = {}


def _emit_body(nc, tc, ctx, d, single_core):
    EXP = mybir.ActivationFunctionType.Exp

    # ---------- persistent pools ----------
    qk_pool = ctx.enter_context(tc.tile_pool(name="qk", bufs=1))
    v_pool = ctx.enter_context(tc.tile_pool(name="v", bufs=1))
    attn_pool = ctx.enter_context(tc.tile_pool(name="attn", bufs=1))
    const_pool = ctx.enter_context(tc.tile_pool(name="const", bufs=1))
    dram_pool = ctx.enter_context(tc.tile_pool(name="dram", bufs=1, space="DRAM"))

    qkT = qk_pool.tile([128, 5 * S], BF16_DT)       # 4 roped+scaled q heads + k
    v_sb = v_pool.tile([128, S], BF16_DT)           # v in [s-part, dh] blocks
    attnT = attn_pool.tile([128, 4 * S], BF16_DT)   # normalized attnT

    ones_col = const_pool.tile([128, 1], BF16_DT)
    ones_row = const_pool.tile([1, 128], BF16_DT)
    ident = const_pool.tile([128, 128], BF16_DT)
    nc.vector.memset(ones_col[:], 1.0)
    nc.vector.memset(ones_row[:], 1.0)

    ag_in = [dram_pool.tile([128, S], BF16_DT, tag=f"agi{h}", name=f"ag_in{h}")
             for h in range(3)]
    ag_out = [dram_pool.tile([512, S], BF16_DT, tag=f"ago{h}", name=f"ag_out{h}")
              for h in range(3)]
    ag_in3 = [dram_pool.tile([128, S // 2], BF16_DT, tag=f"agi3{p}", name=f"ag_in3{p}")
              for p in range(2)]
    ag_out3 = [dram_pool.tile([512, S // 2], BF16_DT, tag=f"ago3{p}", name=f"ag_out3{p}")
               for p in range(2)]
    rs_in = dram_pool.tile([1, HID], BF16_DT)
    rs_out = dram_pool.tile([1, 512], BF16_DT)

    def run_ag(inp, outp):
        if single_core:
            nc.sync.dma_start(outp[0:128, :], inp[:])
        else:
            nc.gpsimd.collective_compute(
                "AllGather", mybir.AluOpType.bypass, replica_groups=GROUPS,
                ins=[inp.opt()], outs=[outp.opt()])

    # ---------- stage A: projections + rope (j-outer) ----------
    with ExitStack() as actx:
        hs_pool = actx.enter_context(tc.tile_pool(name="hs", bufs=1))
        w_pool = actx.enter_context(tc.tile_pool(name="w", bufs=1))
        trig_pool = actx.enter_context(tc.tile_pool(name="trig", bufs=1))
        rope_pool = actx.enter_context(tc.tile_pool(name="rope", bufs=3))

        wq_sb = w_pool.tile([128, NJ * 512], BF16_DT)
        wk_sb = w_pool.tile([128, NJ * 128], BF16_DT)
        wv_sb = w_pool.tile([128, NJ * 128], BF16_DT)
        cos_k = trig_pool.tile([128, S], BF16_DT)
        sin_k = trig_pool.tile([128, S], BF16_DT)
        cos_q = trig_pool.tile([128, S], BF16_DT)
        sin_q = trig_pool.tile([128, S], BF16_DT)
        hsT = hs_pool.tile([128, NJ * S], BF16_DT)

        nc.sync.dma_start(wk_sb[:], d["wk_sb"].ap())
        for j in range(NJ):
            nc.sync.dma_start(hsT[:, j * S:(j + 1) * S], d["hsT"][j])
        nc.sync.dma_start(wv_sb[:], d["wv_sb"].ap())
        nc.sync.dma_start(cos_k[:], d["cos_k"].ap())
        nc.sync.dma_start(sin_k[:], d["sin_k"].ap())
        nc.sync.dma_start(cos_q[:], d["cos_q"].ap())
        nc.sync.dma_start(sin_q[:], d["sin_q"].ap())
        nc.sync.dma_start(ident[:], d["ident"].ap())
        nc.sync.dma_start(wq_sb[:], d["wq_sb"].ap())

        lat0 = w_pool.tile([128, L], BF16_DT)
        wlat = w_pool.tile([128, 128], BF16_DT)
        wlo = w_pool.tile([128, HID], BF16_DT)
        nc.sync.dma_start(lat0[:], d["lat0T"].ap())
        nc.sync.dma_start(wlat[:], d["w_lat"].ap())
        nc.sync.dma_start(wlo[:], d["wlo"].ap())

        def project(pool, w_sb, w_off, w_stride):
            """j-outer projection pass -> 4 psum quarters [128,512]."""
            ps = [pool.tile([128, 512], FP32, tag="proj", name=f"ps{sq}")
                  for sq in range(NSQ)]
            for j in range(NJ):
                for sq in range(NSQ):
                    nc.tensor.matmul(
                        ps[sq][:],
                        lhsT=w_sb[:, w_off + j * w_stride:
                                  w_off + j * w_stride + 128],
                        rhs=hsT[:, j * S + sq * 512: j * S + sq * 512 + 512],
                        start=(j == 0), stop=(j == NJ - 1),
                    )
            return ps

        def rope(ps, dst, dst_off, cosT, sinT):
            for sq in range(NSQ):
                qf = rope_pool.tile([128, 512], FP32, tag="qf")
                nc.scalar.copy(qf[:], ps[sq][:])
                qs = rope_pool.tile([128, 512], FP32, tag="qs")
                nc.sync.dma_start(qs[0:64, :], qf[64:128, :])
                nc.sync.dma_start(qs[64:128, :], qf[0:64, :])
                m1 = rope_pool.tile([128, 512], FP32, tag="m1")
                nc.vector.tensor_mul(m1[:], qf[:], cosT[:, bass.ts(sq, 512)])
                m2 = rope_pool.tile([128, 512], FP32, tag="m2")
                nc.vector.tensor_mul(m2[:], qs[:], sinT[:, bass.ts(sq, 512)])
                nc.vector.tensor_add(
                    dst[:, dst_off + sq * 512: dst_off + sq * 512 + 512],
                    m1[:], m2[:])

        with ExitStack() as pctx:
            pA = pctx.enter_context(tc.tile_pool(name="pA", bufs=8, space="PSUM"))
            pT = pA
            # fused k+v pass: both stream behind the hsT chunk DMAs
            ps_k = [pA.tile([128, 512], FP32, tag="proj", name=f"psk{sq}")
                    for sq in range(NSQ)]
            ps_v = [pA.tile([128, 512], FP32, tag="proj", name=f"psv{sq}")
                    for sq in range(NSQ)]
            for j in range(NJ):
                for ps, w_sb in ((ps_k, wk_sb), (ps_v, wv_sb)):
                    for sq in range(NSQ):
                        nc.tensor.matmul(
                            ps[sq][:],
                            lhsT=w_sb[:, j * 128: j * 128 + 128],
                            rhs=hsT[:, j * S + sq * 512: j * S + sq * 512 + 512],
                            start=(j == 0), stop=(j == NJ - 1),
                        )
            rope(ps_k, qkT, 4 * S, cos_k, sin_k)
            for sq in range(NSQ):
                vT_bf = rope_pool.tile([128, 512], BF16_DT, tag="vbf")
                nc.scalar.copy(vT_bf[:], ps_v[sq][:])
                tp = pT.tile([128, 512], BF16_DT, tag="proj", name="tp")
                for i in range(4):
                    nc.tensor.transpose(
                        tp[:, bass.ts(i, 128)], vT_bf[:, bass.ts(i, 128)],
                        ident[:])
                nc.vector.tensor_copy(v_sb[:, bass.ts(sq, 512)], tp[:])

        # ---------- latent branch (tiny; RS overlaps q projections) ----------
        with ExitStack() as lctx:
            l_pool = lctx.enter_context(tc.tile_pool(name="lat", bufs=1))
            pLs = lctx.enter_context(tc.tile_pool(name="pLs", bufs=2, space="PSUM"))
            pLa = lctx.enter_context(tc.tile_pool(name="pLa", bufs=1, space="PSUM"))

            lp_ps = pLs.tile([128, L], FP32, tag="scL")
            nc.tensor.matmul(lp_ps[:], lhsT=wlat[:], rhs=lat0[:],
                             start=True, stop=True)
            latpT = l_pool.tile([128, L], BF16_DT)
            nc.scalar.copy(latpT[:], lp_ps[:])

            scL = pLs.tile([128, NSB * L], FP32, tag="scL")
            for kc in range(NSB):
                nc.tensor.matmul(
                    scL[:, bass.ts(kc, L)],
                    lhsT=qkT[:, 4 * S + kc * 128: 4 * S + kc * 128 + 128],
                    rhs=latpT[:], start=True, stop=True)
            exL = l_pool.tile([128, NSB * L], BF16_DT, tag="exL")
            nc.scalar.activation(exL[:], scL[:], EXP, scale=SCALE)
            avL = pLa.tile([128, L], FP32, tag="avL")
            denL = pLa.tile([1, L], FP32, tag="denL")
            for kc in range(NSB):
                nc.tensor.matmul(avL[:], lhsT=v_sb[:, bass.ts(kc, 128)],
                                 rhs=exL[:, bass.ts(kc, L)],
                                 start=(kc == 0), stop=(kc == NSB - 1))
                nc.tensor.matmul(denL[:], lhsT=ones_col[:],
                                 rhs=exL[:, bass.ts(kc, L)],
                                 start=(kc == 0), stop=(kc == NSB - 1))
            recipL = l_pool.tile([1, L], BF16_DT, tag="recipL")
            with nc.allow_low_precision(reason="softmax reciprocal in bf16"):
                nc.vector.reciprocal(recipL[:], denL[:])
            bcL = pLs.tile([128, L], FP32, tag="scL")
            nc.tensor.matmul(bcL[:], lhsT=ones_row[:], rhs=recipL[:],
                             start=True, stop=True)
            bcL_sb = l_pool.tile([128, L], FP32, tag="bcLsb")
            nc.vector.tensor_copy(bcL_sb[:], bcL[:])
            attLn = l_pool.tile([128, L], FP32, tag="attLn")
            nc.vector.tensor_mul(attLn[:], avL[:], bcL_sb[:])
            meanv = l_pool.tile([128, 1], FP32, tag="meanv")
            nc.vector.tensor_reduce(meanv[:], attLn[:],
                                    axis=mybir.AxisListType.X,
                                    op=mybir.AluOpType.add)
            nc.vector.tensor_scalar_mul(meanv[:], meanv[:], 1.0 / L)
            meanv_bf = l_pool.tile([128, 1], BF16_DT, tag="meanvbf")
            nc.vector.tensor_copy(meanv_bf[:], meanv[:])
            contrib_sb = l_pool.tile([1, HID], BF16_DT, tag="contrib")
            for i in range(4):
                cps = pLs.tile([1, 512], FP32, tag="scL")
                nc.tensor.matmul(cps[:], lhsT=meanv_bf[:],
                                 rhs=wlo[:, bass.ts(i, 512)],
                                 start=True, stop=True)
                with nc.allow_low_precision(reason="latent contribution bf16"):
                    nc.vector.tensor_copy(contrib_sb[:, bass.ts(i, 512)], cps[:])
            nc.sync.dma_start(rs_in[:], contrib_sb[:])

        if single_core:
            nc.sync.dma_start(rs_out[:], rs_in[:, 0:512])
        else:
            nc.gpsimd.collective_compute(
                "ReduceScatter", mybir.AluOpType.add, replica_groups=GROUPS,
                ins=[rs_in.opt()], outs=[rs_out.opt()])

        with ExitStack() as qctx:
            pA2 = qctx.enter_context(tc.tile_pool(name="pA2", bufs=6, space="PSUM"))
            for hh in range(4):
                rope(project(pA2, wq_sb, hh * 128, 512), qkT, hh * S,
                     cos_q, sin_q)

    # ---------- stage B: attention, flat software pipeline ----------
    with ExitStack() as bctx:
        e_pool = bctx.enter_context(tc.tile_pool(name="expd", bufs=4))
        s_pool = bctx.enter_context(tc.tile_pool(name="s1", bufs=4))
        t_pool = bctx.enter_context(tc.tile_pool(name="tree", bufs=3))
        n_pool = bctx.enter_context(tc.tile_pool(name="norm", bufs=2))
        pS = bctx.enter_context(tc.tile_pool(name="pS", bufs=2, space="PSUM"))
        pAV = bctx.enter_context(tc.tile_pool(name="pAV", bufs=2, space="PSUM"))
        pD = bctx.enter_context(tc.tile_pool(name="pD", bufs=2, space="PSUM"))

        # per-step state (indexed by flat step t = b*8 + k2)
        sc_t = {}    # t -> sc psum tile
        ex_t = {}    # t -> exp'd bf16 tile
        s1_b = {}    # b -> list of 8 pair-sum tiles
        rrow_b = {}  # b -> reciprocal row
        l2_b = {}    # b -> list of L2 tiles
        av_b = {}    # b -> av psum tile
        den_b = {}   # b -> den psum tile
        s4_b = {}    # b -> final tree tile

        def emit_sc(t):
            b, k2 = t // 8, t % 8
            hh, sq = b // 4, b % 4
            sc = pS.tile([128, 1024], FP32, tag="sc")
            for i in range(2):
                kc = k2 * 2 + i
                nc.tensor.matmul(
                    sc[:, bass.ts(i, 512)],
                    lhsT=qkT[:, 4 * S + kc * 128: 4 * S + kc * 128 + 128],
                    rhs=qkT[:, hh * S + sq * 512: hh * S + sq * 512 + 512],
                    start=True, stop=True)
            sc_t[t] = sc

        def emit_exp(t):
            ex = e_pool.tile([128, 1024], BF16_DT, tag="ex")
            nc.scalar.activation(ex[:], sc_t.pop(t)[:], EXP, scale=1.0)
            ex_t[t] = ex

        def emit_av_s1(t):
            b, k2 = t // 8, t % 8
            last = (b == NBLK - 1)
            if k2 == 0:
                av_b[b] = pAV.tile([128, 512], FP32, tag="av", name=f"av{b}")
                s1_b[b] = []
                if last:
                    den_b[b] = pD.tile([1, 512], FP32, tag="db", name="den15")
            ex = ex_t.pop(t)
            for i in range(2):
                kc = k2 * 2 + i
                nc.tensor.matmul(
                    av_b[b][:], lhsT=v_sb[:, bass.ts(kc, 128)],
                    rhs=ex[:, bass.ts(i, 512)],
                    start=(kc == 0), stop=(kc == NSB - 1))
            s1 = s_pool.tile([128, 512], BF16_DT, tag="s1")
            nc.vector.tensor_add(s1[:], ex[:, 0:512], ex[:, 512:1024])
            s1_b[b].append(s1)
            if last:
                # final block: accumulate den directly on the PE so the
                # epilogue doesn't wait for the gpsimd tree
                nc.tensor.matmul(den_b[b][:], lhsT=ones_col[:], rhs=s1[:],
                                 start=(k2 == 0), stop=(k2 == 7))
                return
            # tree adds on gpsimd as pairs become ready
            if k2 in (1, 3, 5, 7):
                if k2 == 1:
                    l2_b[b] = []
                t2 = t_pool.tile([128, 512], BF16_DT, tag="l2")
                nc.gpsimd.tensor_add(t2[:], s1_b[b][k2 - 1][:], s1_b[b][k2][:])
                l2_b[b].append(t2)
            if k2 == 5:
                t3a = t_pool.tile([128, 512], BF16_DT, tag="l3")
                nc.gpsimd.tensor_add(t3a[:], l2_b[b][0][:], l2_b[b][1][:])
                l2_b[b].append(t3a)
            if k2 == 7:
                # l2_b now holds [L2_0, L2_1, L2_2, L3a, L2_3]; finish on DVE
                t3b = t_pool.tile([128, 512], BF16_DT, tag="l3")
                nc.vector.tensor_add(t3b[:], l2_b[b][2][:], l2_b[b][4][:])
                s4 = t_pool.tile([128, 512], BF16_DT, tag="l4")
                nc.vector.tensor_add(s4[:], l2_b[b][3][:], t3b[:])
                s4_b[b] = s4
                del s1_b[b], l2_b[b]

        def emit_den(b):
            den = pD.tile([1, 512], FP32, tag="db")
            nc.tensor.matmul(den[:], lhsT=ones_col[:], rhs=s4_b.pop(b)[:],
                             start=True, stop=True)
            den_b[b] = den

        def emit_recip(b):
            rrow = n_pool.tile([1, 512], BF16_DT, tag="rrow")
            with nc.allow_low_precision(reason="softmax reciprocal in bf16"):
                nc.vector.reciprocal(rrow[:], den_b.pop(b)[:])
            rrow_b[b] = rrow

        def emit_bcmul(b):
            hh, sq = b // 4, b % 4
            bc = pS.tile([128, 512], FP32, tag="sc")
            nc.tensor.matmul(bc[:], lhsT=ones_row[:], rhs=rrow_b.pop(b)[:],
                             start=True, stop=True)
            bc_sb = n_pool.tile([128, 512], FP32, tag="bcsb")
            nc.vector.tensor_copy(bc_sb[:], bc[:])
            nc.vector.tensor_mul(
                attnT[:, hh * S + sq * 512: hh * S + sq * 512 + 512],
                av_b.pop(b)[:], bc_sb[:])

        def ship(hh):
            if hh < 3:
                nc.sync.dma_start(ag_in[hh][:], attnT[:, hh * S:(hh + 1) * S])
                run_ag(ag_in[hh], ag_out[hh])
            elif hh == 3:
                nc.sync.dma_start(ag_in3[0][:], attnT[:, 3 * S: 3 * S + 1024])
                run_ag(ag_in3[0], ag_out3[0])
            else:  # second half of head 3
                nc.sync.dma_start(ag_in3[1][:],
                                  attnT[:, 3 * S + 1024: 3 * S + 2048])
                run_ag(ag_in3[1], ag_out3[1])

        emit_sc(0)
        for t in range(NT):
            b, k2 = t // 8, t % 8
            emit_exp(t)
            if t + 1 < NT:
                emit_sc(t + 1)
            if t >= 1:
                emit_av_s1(t - 1)
            if k2 == 1 and b >= 1:
                emit_den(b - 1)
            if k2 == 2 and b >= 1:
                emit_recip(b - 1)
            if k2 == 5 and b >= 1:
                emit_bcmul(b - 1)
            if k2 == 7 and b >= 1 and (b - 1) % 4 == 3 and b < 13:
                ship((b - 1) // 4)          # heads 0..2 complete & normed
            if k2 == 7 and b == 14:
                ship(3)                     # head-3 first half (sq 0,1 normed)
        emit_av_s1(NT - 1)
        emit_recip(NBLK - 1)
        emit_bcmul(NBLK - 1)
        ship(4)                             # head-3 second half

    # ---------- stage C: output projection (arrival-ordered) ----------
    with ExitStack() as cctx:
        g_pool = cctx.enter_context(tc.tile_pool(name="gath", bufs=2))
        wo_pool = cctx.enter_context(tc.tile_pool(name="wo", bufs=1))
        oa_pool = cctx.enter_context(tc.tile_pool(name="oacc", bufs=1))
        o_pool = cctx.enter_context(tc.tile_pool(name="oev", bufs=3))
        pO = cctx.enter_context(tc.tile_pool(name="pO", bufs=3, space="PSUM"))

        wo_sb = wo_pool.tile([128, NJ * 512], BF16_DT)
        nc.sync.dma_start(wo_sb[:], d["wo_sb"].ap())
        latrow = wo_pool.tile([1, 512], BF16_DT)
        nc.sync.dma_start(latrow[:], rs_out[:])

        o_acc = oa_pool.tile([128, NSB * 512], FP32)

        # arrival order: heads 0..2 (whole), then head-3 half 0 / half 1
        parts = [(0, None), (1, None), (2, None), (3, 0), (3, 1)]
        for a, half in parts:
            if half is None:
                garr = g_pool.tile([128, 4 * S], BF16_DT, tag="garr",
                                   name=f"garr{a}")
                for r in range(4):
                    nc.sync.dma_start(garr[:, r * S:(r + 1) * S],
                                      ag_out[a][r * 128:(r + 1) * 128, :])
                sbs = range(NSB)
                cw = S
            else:
                garr = g_pool.tile([128, 4 * 1024], BF16_DT, tag="garr",
                                   name=f"garr3{half}")
                for r in range(4):
                    nc.sync.dma_start(
                        garr[:, r * 1024:(r + 1) * 1024],
                        ag_out3[half][r * 128:(r + 1) * 128, :])
                sbs = range(half * 8, half * 8 + 8)
                cw = 1024
            for sb in sbs:
                sbo = sb - (half * 8 if half is not None else 0)
                ops = pO.tile([128, 512], FP32, tag="ops")
                for r in range(4):
                    j = 4 * r + a
                    nc.tensor.matmul(
                        ops[:],
                        lhsT=garr[:, r * cw + sbo * 128: r * cw + sbo * 128 + 128],
                        rhs=wo_sb[:, bass.ts(j, 512)],
                        start=(r == 0), stop=(r == 3 and a != 3))
                if a == 3:
                    nc.tensor.matmul(ops[:], lhsT=ones_row[:],
                                     rhs=latrow[:], start=False, stop=True)
                acc_sl = o_acc[:, sb * 512:(sb + 1) * 512]
                if a == 0:
                    nc.vector.tensor_copy(acc_sl, ops[:])
                elif a < 3:
                    nc.vector.tensor_add(acc_sl, acc_sl, ops[:])
                else:
                    oev = o_pool.tile([128, 512], FP32, tag="oev")
                    nc.vector.tensor_add(oev[:], acc_sl, ops[:])
                    nc.sync.dma_start(
                        d["y"].ap()[sb * 128:(sb + 1) * 128, :], oev[:])


def _build_kernel(reps=1, single_core=False):
    nc = bacc.Bacc("TRN2", target_bir_lowering=False, debug=False,
                   num_devices=(1 if single_core else N_CORES))

    d = {
        "hsT": nc.dram_tensor("hsT", [NJ, 128, S], BF16_DT, kind="ExternalInput"),
        "wq_sb": nc.dram_tensor("wq_sb", [128, NJ * 512], BF16_DT, kind="ExternalInput"),
        "wk_sb": nc.dram_tensor("wk_sb", [128, NJ * 128], BF16_DT, kind="ExternalInput"),
        "wv_sb": nc.dram_tensor("wv_sb", [128, NJ * 128], BF16_DT, kind="ExternalInput"),
        "wo_sb": nc.dram_tensor("wo_sb", [128, NJ * 512], BF16_DT, kind="ExternalInput"),
        "cos_k": nc.dram_tensor("cos_k", [128, S], BF16_DT, kind="ExternalInput"),
        "sin_k": nc.dram_tensor("sin_k", [128, S], BF16_DT, kind="ExternalInput"),
        "cos_q": nc.dram_tensor("cos_q", [128, S], BF16_DT, kind="ExternalInput"),
        "sin_q": nc.dram_tensor("sin_q", [128, S], BF16_DT, kind="ExternalInput"),
        "ident": nc.dram_tensor("ident", [128, 128], BF16_DT, kind="ExternalInput"),
        "lat0T": nc.dram_tensor("lat0T", [128, L], BF16_DT, kind="ExternalInput"),
        "w_lat": nc.dram_tensor("w_lat", [128, 128], BF16_DT, kind="ExternalInput"),
        "wlo": nc.dram_tensor("wlo", [128, HID], BF16_DT, kind="ExternalInput"),
        "y": nc.dram_tensor("y", [S, 512], FP32, kind="ExternalOutput"),
    }

    with tile.TileContext(nc) as tc:
        with ExitStack() as ctx:
            _emit_body(nc, tc, ctx, d, single_core)

    nc.compile()
    return nc


def _host_inputs(hs, latent, w_latent, wq, wk, wv, wo, wlo):
    """Build the 8 per-core input maps."""
    inv_freq = 1.0 / (THETA ** (np.arange(0, DH, 2, dtype=np.float32) / DH))
    t = np.arange(S, dtype=np.float32)
    freqs = np.outer(t, inv_freq)
    emb = np.concatenate([freqs, freqs], axis=-1)          # [S, DH]
    cosT = np.ascontiguousarray(np.cos(emb).T.astype(np.float32))
    sinT = np.sin(emb).T.astype(np.float32)
    sinS = sinT.copy()
    sinS[:64] *= -1.0
    cos_k = cosT.astype(BF16)
    sin_k = np.ascontiguousarray(sinS).astype(BF16)
    cos_q = (cosT * SCALE).astype(BF16)
    sin_q = (sinS * SCALE).astype(BF16)
    ident = np.eye(128, dtype=BF16)

    def chunked(w, cols):
        return np.ascontiguousarray(
            w.reshape(NJ, 128, cols).transpose(1, 0, 2).reshape(128, NJ * cols)
        ).astype(BF16)

    in_maps = []
    for c in range(N_CORES):
        b, g = c // 4, c % 4
        hsT = np.ascontiguousarray(hs[b].T).astype(BF16).reshape(NJ, 128, S)
        in_maps.append({
            "hsT": hsT,
            "wq_sb": chunked(wq[:, 4 * g * DH:(4 * g + 4) * DH], 512),
            "wk_sb": chunked(wk[:, g * DH:(g + 1) * DH], 128),
            "wv_sb": chunked(wv[:, g * DH:(g + 1) * DH], 128),
            "wo_sb": chunked(wo[:, g * 512:(g + 1) * 512], 512),
            "cos_k": cos_k, "sin_k": sin_k,
            "cos_q": cos_q, "sin_q": sin_q,
            "ident": ident,
            "lat0T": np.ascontiguousarray(latent[0, g].T).astype(BF16),
            "w_lat": w_latent.astype(BF16),
            "wlo": np.ascontiguousarray(wlo[g * DH:(g + 1) * DH, :]).astype(BF16),
        })
    return in_maps


def kernel(hidden_states, latent, w_latent, wq, wk, wv, wo, w_latent_o,
           _trace=False, _trace_cores=None, _tmpdir=None):
    hs = np.asarray(hidden_states, np.float32)
    in_maps = _host_inputs(hs, np.asarray(latent), np.asarray(w_latent),
                           np.asarray(wq), np.asarray(wk), np.asarray(wv),
                           np.asarray(wo), np.asarray(w_latent_o))
    if "nc" not in _COMPILED:
        _COMPILED["nc"] = _build_kernel()
    nc = _COMPILED["nc"]
    res = run_bass_kernel_spmd(nc, in_maps, list(range(N_CORES)),
                               trace=_trace, trace_cores=_trace_cores,
                               tmpdir=_tmpdir)
    kernel.last_result = res
    out = np.empty((B, S, HID), np.float32)
    for c in range(N_CORES):
        b, g = c // 4, c % 4
        out[b, :, g * 512:(g + 1) * 512] = res.results[c]["y"]
    return out


kernel.last_result = None


# revision 18
# speedup vs baseline: 1.0831x; 1.0831x over previous
"""MLA (multi-head latent attention) Trainium2 kernel, 8-way tensor/data parallel.

Problem shapes (hardcoded): B=2, S=2048, HID=2048, H=16, KVH=4, DH=128, L=64.

Sharding: core c -> batch b = c//4, kv-group g = c%4.
Each core computes q-heads 4g..4g+3 and kv head g for its batch.

Stage A: q/k/v projections j-outer (4 seq-quarter PSUM accumulators per pass,
  so consecutive matmuls share the stationary weight chunk), RoPE on DVE with
  bf16 trig (q trig pre-scaled by 1/sqrt(DH) so attention exp needs no scale),
  j-major hsT DMA so the k pass streams behind the DMA; latent branch last.
Stage B: flat software pipeline over all 64 (head, sq, k2) steps:
  exp(t) | scores(t+1) | av(t-1) | pair-sum(t-1), with the softmax denominator
  reduced by a DVE/GPSIMD add tree to one [128,512] tile -> single ones-matmul,
  reciprocal on DVE (PSUM src, bf16 out), broadcast matmul in bf16, and the
  whole normalization of block b deferred into block b+1 so the PE never
  stalls.  AllGather per head; head 3 in two halves.
Stage C: o-projection consumes gathered heads in ARRIVAL order (j-outer),
  parking partial sums in SBUF f32; only head-3's matmuls trail the last
  AllGather; latent contribution (ReduceScattered earlier) folded in as a
  bf16 rank-1 update.
"""

import numpy as np
import ml_dtypes
from contextlib import ExitStack

import concourse.bass as bass
import concourse.bacc as bacc
import concourse.tile as tile
import concourse.mybir as mybir
from concourse.bass_utils import run_bass_kernel_spmd

BF16 = ml_dtypes.bfloat16
FP32 = mybir.dt.float32
BF16_DT = mybir.dt.bfloat16

B, S, HID = 2, 2048, 2048
H, KVH, DH, L = 16, 4, 128, 64
THETA = 10000.0
N_CORES = 8
GROUPS = [[0, 1, 2, 3], [4, 5, 6, 7]]
NJ = HID // 128          # 16 contraction chunks
NSQ = S // 512           # 4 seq blocks of 512
NSB = S // 128           # 16 seq blocks of 128
SCALE = 1.0 / float(np.sqrt(np.float32(DH)))
NBLK = 16                # 4 heads x 4 sq blocks
NT = NBLK * 8            # flat pipeline steps (one per k2 chunk-pair)

_COMPILED = {}


def _emit_body(nc, tc, ctx, d, single_core):
    EXP = mybir.ActivationFunctionType.Exp

    # ---------- persistent pools ----------
    qk_pool = ctx.enter_context(tc.tile_pool(name="qk", bufs=1))
    v_pool = ctx.enter_context(tc.tile_pool(name="v", bufs=1))
    attn_pool = ctx.enter_context(tc.tile_pool(name="attn", bufs=1))
    const_pool = ctx.enter_context(tc.tile_pool(name="const", bufs=1))
    dram_pool = ctx.enter_context(tc.tile_pool(name="dram", bufs=1, space="DRAM"))

    qkT = qk_pool.tile([128, 5 * S], BF16_DT)       # 4 roped+scaled q heads + k
    v_sb = v_pool.tile([128, S], BF16_DT)           # v in [s-part, dh] blocks
    attnT = attn_pool.tile([128, 4 * S], BF16_DT)   # normalized attnT

    ones_col = const_pool.tile([128, 1], BF16_DT)
    ones_row = const_pool.tile([1, 128], BF16_DT)
    ident = const_pool.tile([128, 128], BF16_DT)
    nc.vector.memset(ones_col[:], 1.0)
    nc.vector.memset(ones_row[:], 1.0)

    ag_in = [dram_pool.tile([128, S], BF16_DT, tag=f"agi{h}", name=f"ag_in{h}")
             for h in range(3)]
    ag_out = [dram_pool.tile([512, S], BF16_DT, tag=f"ago{h}", name=f"ag_out{h}")
              for h in range(3)]
    ag_in3 = [dram_pool.tile([128, S // 2], BF16_DT, tag=f"agi3{p}", name=f"ag_in3{p}")
              for p in range(2)]
    ag_out3 = [dram_pool.tile([512, S // 2], BF16_DT, tag=f"ago3{p}", name=f"ag_out3{p}")
               for p in range(2)]
    rs_in = dram_pool.tile([1, HID], BF16_DT)
    rs_out = dram_pool.tile([1, 512], BF16_DT)

    def run_ag(inp, outp):
        if single_core:
            nc.sync.dma_start(outp[0:128, :], inp[:])
        else:
            nc.gpsimd.collective_compute(
                "AllGather", mybir.AluOpType.bypass, replica_groups=GROUPS,
                ins=[inp.opt()], outs=[outp.opt()])

    # ---------- stage A: projections + rope (j-outer) ----------
    with ExitStack() as actx:
        hs_pool = actx.enter_context(tc.tile_pool(name="hs", bufs=1))
        w_pool = actx.enter_context(tc.tile_pool(name="w", bufs=1))
        trig_pool = actx.enter_context(tc.tile_pool(name="trig", bufs=1))
        rope_pool = actx.enter_context(tc.tile_pool(name="rope", bufs=3))

        wq_sb = w_pool.tile([128, NJ * 512], BF16_DT)
        wk_sb = w_pool.tile([128, NJ * 128], BF16_DT)
        wv_sb = w_pool.tile([128, NJ * 128], BF16_DT)
        cos_k = trig_pool.tile([128, S], BF16_DT)
        sin_k = trig_pool.tile([128, S], BF16_DT)
        cos_q = trig_pool.tile([128, S], BF16_DT)
        sin_q = trig_pool.tile([128, S], BF16_DT)
        hsT = hs_pool.tile([128, NJ * S], BF16_DT)

        nc.sync.dma_start(wk_sb[:], d["wk_sb"].ap())
        for j in range(NJ):
            nc.sync.dma_start(hsT[:, j * S:(j + 1) * S], d["hsT"][j])
        nc.sync.dma_start(wv_sb[:], d["wv_sb"].ap())
        nc.sync.dma_start(cos_k[:], d["cos_k"].ap())
        nc.sync.dma_start(sin_k[:], d["sin_k"].ap())
        nc.sync.dma_start(cos_q[:], d["cos_q"].ap())
        nc.sync.dma_start(sin_q[:], d["sin_q"].ap())
        nc.sync.dma_start(ident[:], d["ident"].ap())
        nc.sync.dma_start(wq_sb[:], d["wq_sb"].ap())

        lat0 = w_pool.tile([128, L], BF16_DT)
        wlat = w_pool.tile([128, 128], BF16_DT)
        wlo = w_pool.tile([128, HID], BF16_DT)
        nc.sync.dma_start(lat0[:], d["lat0T"].ap())
        nc.sync.dma_start(wlat[:], d["w_lat"].ap())
        nc.sync.dma_start(wlo[:], d["wlo"].ap())

        def project(pool, w_sb, w_off, w_stride):
            """j-outer projection pass -> 4 psum quarters [128,512]."""
            ps = [pool.tile([128, 512], FP32, tag="proj", name=f"ps{sq}")
                  for sq in range(NSQ)]
            for j in range(NJ):
                for sq in range(NSQ):
                    nc.tensor.matmul(
                        ps[sq][:],
                        lhsT=w_sb[:, w_off + j * w_stride:
                                  w_off + j * w_stride + 128],
                        rhs=hsT[:, j * S + sq * 512: j * S + sq * 512 + 512],
                        start=(j == 0), stop=(j == NJ - 1),
                    )
            return ps

        def rope(ps, dst, dst_off, cosT, sinT):
            for sq in range(NSQ):
                qf = rope_pool.tile([128, 512], FP32, tag="qf")
                nc.scalar.copy(qf[:], ps[sq][:])
                qs = rope_pool.tile([128, 512], FP32, tag="qs")
                nc.sync.dma_start(qs[0:64, :], qf[64:128, :])
                nc.sync.dma_start(qs[64:128, :], qf[0:64, :])
                m1 = rope_pool.tile([128, 512], FP32, tag="m1")
                nc.vector.tensor_mul(m1[:], qf[:], cosT[:, bass.ts(sq, 512)])
                m2 = rope_pool.tile([128, 512], FP32, tag="m2")
                nc.vector.tensor_mul(m2[:], qs[:], sinT[:, bass.ts(sq, 512)])
                nc.vector.tensor_add(
                    dst[:, dst_off + sq * 512: dst_off + sq * 512 + 512],
                    m1[:], m2[:])

        with ExitStack() as pctx:
            pA = pctx.enter_context(tc.tile_pool(name="pA", bufs=8, space="PSUM"))
            pT = pA
            # fused k+v pass: both stream behind the hsT chunk DMAs
            ps_k = [pA.tile([128, 512], FP32, tag="proj", name=f"psk{sq}")
                    for sq in range(NSQ)]
            ps_v = [pA.tile([128, 512], FP32, tag="proj", name=f"psv{sq}")
                    for sq in range(NSQ)]
            for j in range(NJ):
                for ps, w_sb in ((ps_k, wk_sb), (ps_v, wv_sb)):
                    for sq in range(NSQ):
                        nc.tensor.matmul(
                            ps[sq][:],
                            lhsT=w_sb[:, j * 128: j * 128 + 128],
                            rhs=hsT[:, j * S + sq * 512: j * S + sq * 512 + 512],
                            start=(j == 0), stop=(j == NJ - 1),
                        )
            rope(ps_k, qkT, 4 * S, cos_k, sin_k)
            for sq in range(NSQ):
                vT_bf = rope_pool.tile([128, 512], BF16_DT, tag="vbf")
                nc.scalar.copy(vT_bf[:], ps_v[sq][:])
                tp = pT.tile([128, 512], BF16_DT, tag="proj", name="tp")
                for i in range(4):
                    nc.tensor.transpose(
                        tp[:, bass.ts(i, 128)], vT_bf[:, bass.ts(i, 128)],
                        ident[:])
                nc.vector.tensor_copy(v_sb[:, bass.ts(sq, 512)], tp[:])

        # ---------- latent branch (tiny; RS overlaps q projections) ----------
        with ExitStack() as lctx:
            l_pool = lctx.enter_context(tc.tile_pool(name="lat", bufs=1))
            pLs = lctx.enter_context(tc.tile_pool(name="pLs", bufs=2, space="PSUM"))
            pLa = lctx.enter_context(tc.tile_pool(name="pLa", bufs=1, space="PSUM"))

            lp_ps = pLs.tile([128, L], FP32, tag="scL")
            nc.tensor.matmul(lp_ps[:], lhsT=wlat[:], rhs=lat0[:],
                             start=True, stop=True)
            latpT = l_pool.tile([128, L], BF16_DT)
            nc.scalar.copy(latpT[:], lp_ps[:])

            scL = pLs.tile([128, NSB * L], FP32, tag="scL")
            for kc in range(NSB):
                nc.tensor.matmul(
                    scL[:, bass.ts(kc, L)],
                    lhsT=qkT[:, 4 * S + kc * 128: 4 * S + kc * 128 + 128],
                    rhs=latpT[:], start=True, stop=True)
            exL = l_pool.tile([128, NSB * L], BF16_DT, tag="exL")
            nc.scalar.activation(exL[:], scL[:], EXP, scale=SCALE)
            avL = pLa.tile([128, L], FP32, tag="avL")
            denL = pLa.tile([1, L], FP32, tag="denL")
            for kc in range(NSB):
                nc.tensor.matmul(avL[:], lhsT=v_sb[:, bass.ts(kc, 128)],
                                 rhs=exL[:, bass.ts(kc, L)],
                                 start=(kc == 0), stop=(kc == NSB - 1))
                nc.tensor.matmul(denL[:], lhsT=ones_col[:],
                                 rhs=exL[:, bass.ts(kc, L)],
                                 start=(kc == 0), stop=(kc == NSB - 1))
            recipL = l_pool.tile([1, L], BF16_DT, tag="recipL")
            with nc.allow_low_precision(reason="softmax reciprocal in bf16"):
                nc.vector.reciprocal(recipL[:], denL[:])
            bcL = pLs.tile([128, L], FP32, tag="scL")
            nc.tensor.matmul(bcL[:], lhsT=ones_row[:], rhs=recipL[:],
                             start=True, stop=True)
            bcL_sb = l_pool.tile([128, L], FP32, tag="bcLsb")
            nc.vector.tensor_copy(bcL_sb[:], bcL[:])
            attLn = l_pool.tile([128, L], FP32, tag="attLn")
            nc.vector.tensor_mul(attLn[:], avL[:], bcL_sb[:])
            meanv = l_pool.tile([128, 1], FP32, tag="meanv")
            nc.vector.tensor_reduce(meanv[:], attLn[:],
                                    axis=mybir.AxisListType.X,
                                    op=mybir.AluOpType.add)
            nc.vector.tensor_scalar_mul(meanv[:], meanv[:], 1.0 / L)
            meanv_bf = l_pool.tile([128, 1], BF16_DT, tag="meanvbf")
            nc.vector.tensor_copy(meanv_bf[:], meanv[:])
            contrib_sb = l_pool.tile([1, HID], BF16_DT, tag="contrib")
            for i in range(4):
                cps = pLs.tile([1, 512], FP32, tag="scL")
                nc.tensor.matmul(cps[:], lhsT=meanv_bf[:],
                                 rhs=wlo[:, bass.ts(i, 512)],
                                 start=True, stop=True)
                with nc.allow_low_precision(reason="latent contribution bf16"):
                    nc.vector.tensor_copy(contrib_sb[:, bass.ts(i, 512)], cps[:])
            nc.sync.dma_start(rs_in[:], contrib_sb[:])

        if single_core:
            nc.sync.dma_start(rs_out[:], rs_in[:, 0:512])
        else:
            nc.gpsimd.collective_compute(
                "ReduceScatter", mybir.AluOpType.add, replica_groups=GROUPS,
                ins=[rs_in.opt()], outs=[rs_out.opt()])

        with ExitStack() as qctx:
            pA2 = qctx.enter_context(tc.tile_pool(name="pA2", bufs=6, space="PSUM"))
            for hh in range(4):
                rope(project(pA2, wq_sb, hh * 128, 512), qkT, hh * S,
                     cos_q, sin_q)

    # ---------- stage B: attention, flat software pipeline ----------
    with ExitStack() as bctx:
        e_pool = bctx.enter_context(tc.tile_pool(name="expd", bufs=4))
        s_pool = bctx.enter_context(tc.tile_pool(name="s1", bufs=4))
        t_pool = bctx.enter_context(tc.tile_pool(name="tree", bufs=3))
        n_pool = bctx.enter_context(tc.tile_pool(name="norm", bufs=2))
        pS = bctx.enter_context(tc.tile_pool(name="pS", bufs=2, space="PSUM"))
        pAV = bctx.enter_context(tc.tile_pool(name="pAV", bufs=2, space="PSUM"))
        pD = bctx.enter_context(tc.tile_pool(name="pD", bufs=2, space="PSUM"))

        # per-step state (indexed by flat step t = b*8 + k2)
        sc_t = {}    # t -> sc psum tile
        ex_t = {}    # t -> exp'd bf16 tile
        s1_b = {}    # b -> list of 8 pair-sum tiles
        rrow_b = {}  # b -> reciprocal row
        l2_b = {}    # b -> list of L2 tiles
        av_b = {}    # b -> av psum tile
        den_b = {}   # b -> den psum tile
        s4_b = {}    # b -> final tree tile

        def emit_sc(t):
            b, k2 = t // 8, t % 8
            hh, sq = b // 4, b % 4
            sc = pS.tile([128, 1024], FP32, tag="sc")
            for i in range(2):
                kc = k2 * 2 + i
                nc.tensor.matmul(
                    sc[:, bass.ts(i, 512)],
                    lhsT=qkT[:, 4 * S + kc * 128: 4 * S + kc * 128 + 128],
                    rhs=qkT[:, hh * S + sq * 512: hh * S + sq * 512 + 512],
                    start=True, stop=True)
            sc_t[t] = sc

        def emit_exp(t):
            ex = e_pool.tile([128, 1024], BF16_DT, tag="ex")
            nc.scalar.activation(ex[:], sc_t.pop(t)[:], EXP, scale=1.0)
            ex_t[t] = ex

        def emit_av_s1(t):
            b, k2 = t // 8, t % 8
            last = (b == NBLK - 1)
            if k2 == 0:
                av_b[b] = pAV.tile([128, 512], FP32, tag="av", name=f"av{b}")
                s1_b[b] = []
                if last:
                    den_b[b] = pD.tile([1, 512], FP32, tag="db", name="den15")
            ex = ex_t.pop(t)
            for i in range(2):
                kc = k2 * 2 + i
                nc.tensor.matmul(
                    av_b[b][:], lhsT=v_sb[:, bass.ts(kc, 128)],
                    rhs=ex[:, bass.ts(i, 512)],
                    start=(kc == 0), stop=(kc == NSB - 1))
            s1 = s_pool.tile([128, 512], BF16_DT, tag="s1")
            nc.vector.tensor_add(s1[:], ex[:, 0:512], ex[:, 512:1024])
            s1_b[b].append(s1)
            if last:
                # final block: accumulate den directly on the PE so the
                # epilogue doesn't wait for the gpsimd tree
                nc.tensor.matmul(den_b[b][:], lhsT=ones_col[:], rhs=s1[:],
                                 start=(k2 == 0), stop=(k2 == 7))
                return
            # tree adds on gpsimd as pairs become ready
            if k2 in (1, 3, 5, 7):
                if k2 == 1:
                    l2_b[b] = []
                t2 = t_pool.tile([128, 512], BF16_DT, tag="l2")
                nc.gpsimd.tensor_add(t2[:], s1_b[b][k2 - 1][:], s1_b[b][k2][:])
                l2_b[b].append(t2)
            if k2 == 5:
                t3a = t_pool.tile([128, 512], BF16_DT, tag="l3")
                nc.gpsimd.tensor_add(t3a[:], l2_b[b][0][:], l2_b[b][1][:])
                l2_b[b].append(t3a)
            if k2 == 7:
                # l2_b now holds [L2_0, L2_1, L2_2, L3a, L2_3]; finish on DVE
                t3b = t_pool.tile([128, 512], BF16_DT, tag="l3")
                nc.vector.tensor_add(t3b[:], l2_b[b][2][:], l2_b[b][4][:])
                s4 = t_pool.tile([128, 512], BF16_DT, tag="l4")
                nc.vector.tensor_add(s4[:], l2_b[b][3][:], t3b[:])
                s4_b[b] = s4
                del s1_b[b], l2_b[b]

        def emit_den(b):
            den = pD.tile([1, 512], FP32, tag="db")
            nc.tensor.matmul(den[:], lhsT=ones_col[:], rhs=s4_b.pop(b)[:],
                             start=True, stop=True)
            den_b[b] = den

        def emit_recip(b):
            rrow = n_pool.tile([1, 512], BF16_DT, tag="rrow")
            with nc.allow_low_precision(reason="softmax reciprocal in bf16"):
                nc.vector.reciprocal(rrow[:], den_b.pop(b)[:])
            rrow_b[b] = rrow

        def emit_bcmul(b):
            hh, sq = b // 4, b % 4
            bc = pS.tile([128, 512], FP32, tag="sc")
            nc.tensor.matmul(bc[:], lhsT=ones_row[:], rhs=rrow_b.pop(b)[:],
                             start=True, stop=True)
            bc_sb = n_pool.tile([128, 512], FP32, tag="bcsb")
            nc.vector.tensor_copy(bc_sb[:], bc[:])
            nc.vector.tensor_mul(
                attnT[:, hh * S + sq * 512: hh * S + sq * 512 + 512],
                av_b.pop(b)[:], bc_sb[:])

        def ship(hh):
            if hh < 3:
                nc.sync.dma_start(ag_in[hh][:], attnT[:, hh * S:(hh + 1) * S])
                run_ag(ag_in[hh], ag_out[hh])
            elif hh == 3:
                nc.sync.dma_start(ag_in3[0][:], attnT[:, 3 * S: 3 * S + 1024])
                run_ag(ag_in3[0], ag_out3[0])
            else:  # second half of head 3
                nc.sync.dma_start(ag_in3[1][:],
                                  attnT[:, 3 * S + 1024: 3 * S + 2048])
                run_ag(ag_in3[1], ag_out3[1])

        emit_sc(0)
        for t in range(NT):
            b, k2 = t // 8, t % 8
            emit_exp(t)
            if t + 1 < NT:
                emit_sc(t + 1)
            if t >= 1:
                emit_av_s1(t - 1)
            if k2 == 2 and b >= 1:
                emit_den(b - 1)
            if k2 == 3 and b >= 1:
                emit_recip(b - 1)
            if k2 == 6 and b >= 1:
                emit_bcmul(b - 1)
            if k2 == 7 and b >= 1 and (b - 1) % 4 == 3 and b < 13:
                ship((b - 1) // 4)          # heads 0..2 complete & normed
            if k2 == 7 and b == 14:
                ship(3)                     # head-3 first half (sq 0,1 normed)
        emit_av_s1(NT - 1)
        emit_recip(NBLK - 1)
        emit_bcmul(NBLK - 1)
        ship(4)                             # head-3 second half

    # ---------- stage C: output projection (arrival-ordered) ----------
    with ExitStack() as cctx:
        g_pool = cctx.enter_context(tc.tile_pool(name="gath", bufs=2))
        wo_pool = cctx.enter_context(tc.tile_pool(name="wo", bufs=1))
        oa_pool = cctx.enter_context(tc.tile_pool(name="oacc", bufs=1))
        o_pool = cctx.enter_context(tc.tile_pool(name="oev", bufs=3))
        pO = cctx.enter_context(tc.tile_pool(name="pO", bufs=3, space="PSUM"))

        wo_sb = wo_pool.tile([128, NJ * 512], BF16_DT)
        nc.sync.dma_start(wo_sb[:], d["wo_sb"].ap())
        latrow = wo_pool.tile([1, 512], BF16_DT)
        nc.sync.dma_start(latrow[:], rs_out[:])

        o_acc = oa_pool.tile([128, NSB * 512], FP32)

        # arrival order: heads 0..2 (whole), then head-3 half 0 / half 1
        parts = [(0, None), (1, None), (2, None), (3, 0), (3, 1)]
        for a, half in parts:
            if half is None:
                garr = g_pool.tile([128, 4 * S], BF16_DT, tag="garr",
                                   name=f"garr{a}")
                for r in range(4):
                    nc.sync.dma_start(garr[:, r * S:(r + 1) * S],
                                      ag_out[a][r * 128:(r + 1) * 128, :])
                sbs = range(NSB)
                cw = S
            else:
                garr = g_pool.tile([128, 4 * 1024], BF16_DT, tag="garr",
                                   name=f"garr3{half}")
                for r in range(4):
                    nc.sync.dma_start(
                        garr[:, r * 1024:(r + 1) * 1024],
                        ag_out3[half][r * 128:(r + 1) * 128, :])
                sbs = range(half * 8, half * 8 + 8)
                cw = 1024
            for sb in sbs:
                sbo = sb - (half * 8 if half is not None else 0)
                ops = pO.tile([128, 512], FP32, tag="ops")
                for r in range(4):
                    j = 4 * r + a
                    nc.tensor.matmul(
                        ops[:],
                        lhsT=garr[:, r * cw + sbo * 128: r * cw + sbo * 128 + 128],
                        rhs=wo_sb[:, bass.ts(j, 512)],
                        start=(r == 0), stop=(r == 3 and a != 3))
                if a == 3:
                    nc.tensor.matmul(ops[:], lhsT=ones_row[:],
                                     rhs=latrow[:], start=False, stop=True)
                acc_sl = o_acc[:, sb * 512:(sb + 1) * 512]
                if a == 0:
                    nc.vector.tensor_copy(acc_sl, ops[:])
                elif a < 3:
                    nc.vector.tensor_add(acc_sl, acc_sl, ops[:])
                else:
                    oev = o_pool.tile([128, 512], FP32, tag="oev")
                    nc.vector.tensor_add(oev[:], acc_sl, ops[:])
                    nc.sync.dma_start(
                        d["y"].ap()[sb * 128:(sb + 1) * 128, :], oev[:])


def _build_kernel(reps=1, single_core=False):
    nc = bacc.Bacc("TRN2", target_bir_lowering=False, debug=False,
                   num_devices=(1 if single_core else N_CORES))

    d = {
        "hsT": nc.dram_tensor("hsT", [NJ, 128, S], BF16_DT, kind="ExternalInput"),
        "wq_sb": nc.dram_tensor("wq_sb", [128, NJ * 512], BF16_DT, kind="ExternalInput"),
        "wk_sb": nc.dram_tensor("wk_sb", [128, NJ * 128], BF16_DT, kind="ExternalInput"),
        "wv_sb": nc.dram_tensor("wv_sb", [128, NJ * 128], BF16_DT, kind="ExternalInput"),
        "wo_sb": nc.dram_tensor("wo_sb", [128, NJ * 512], BF16_DT, kind="ExternalInput"),
        "cos_k": nc.dram_tensor("cos_k", [128, S], BF16_DT, kind="ExternalInput"),
        "sin_k": nc.dram_tensor("sin_k", [128, S], BF16_DT, kind="ExternalInput"),
        "cos_q": nc.dram_tensor("cos_q", [128, S], BF16_DT, kind="ExternalInput"),
        "sin_q": nc.dram_tensor("sin_q", [128, S], BF16_DT, kind="ExternalInput"),
        "ident": nc.dram_tensor("ident", [128, 128], BF16_DT, kind="ExternalInput"),
        "lat0T": nc.dram_tensor("lat0T", [128, L], BF16_DT, kind="ExternalInput"),
        "w_lat": nc.dram_tensor("w_lat", [128, 128], BF16_DT, kind="ExternalInput"),
        "wlo": nc.dram_tensor("wlo", [128, HID], BF16_DT, kind="ExternalInput"),
        "y": nc.dram_tensor("y", [S, 512], FP32, kind="ExternalOutput"),
    }

    with tile.TileContext(nc) as tc:
        with ExitStack() as ctx:
            _emit_body(nc, tc, ctx, d, single_core)

    nc.compile()
    return nc


def _host_inputs(hs, latent, w_latent, wq, wk, wv, wo, wlo):
    """Build the 8 per-core input maps."""
    inv_freq = 1.0 / (THETA ** (np.arange(0, DH, 2, dtype=np.float32) / DH))
    t = np.arange(S, dtype=np.float32)
    freqs = np.outer(t, inv_freq)
    emb = np.concatenate([freqs, freqs], axis=-1)          # [S, DH]
    cosT = np.ascontiguousarray(np.cos(emb).T.astype(np.float32))
    sinT = np.sin(emb).T.astype(np.float32)
    sinS = sinT.copy()
    sinS[:64] *= -1.0
    cos_k = cosT.astype(BF16)
    sin_k = np.ascontiguousarray(sinS).astype(BF16)
    cos_q = (cosT * SCALE).astype(BF16)
    sin_q = (sinS * SCALE).astype(BF16)
    ident = np.eye(128, dtype=BF16)

    def chunked(w, cols):
        return np.ascontiguousarray(
            w.reshape(NJ, 128, cols).transpose(1, 0, 2).reshape(128, NJ * cols)
        ).astype(BF16)

    in_maps = []
    for c in range(N_CORES):
        b, g = c // 4, c % 4
        hsT = np.ascontiguousarray(hs[b].T).astype(BF16).reshape(NJ, 128, S)
        in_maps.append({
            "hsT": hsT,
            "wq_sb": chunked(wq[:, 4 * g * DH:(4 * g + 4) * DH], 512),
            "wk_sb": chunked(wk[:, g * DH:(g + 1) * DH], 128),
            "wv_sb": chunked(wv[:, g * DH:(g + 1) * DH], 128),
            "wo_sb": chunked(wo[:, g * 512:(g + 1) * 512], 512),
            "cos_k": cos_k, "sin_k": sin_k,
            "cos_q": cos_q, "sin_q": sin_q,
            "ident": ident,
            "lat0T": np.ascontiguousarray(latent[0, g].T).astype(BF16),
            "w_lat": w_latent.astype(BF16),
            "wlo": np.ascontiguousarray(wlo[g * DH:(g + 1) * DH, :]).astype(BF16),
        })
    return in_maps


def kernel(hidden_states, latent, w_latent, wq, wk, wv, wo, w_latent_o,
           _trace=False, _trace_cores=None, _tmpdir=None):
    hs = np.asarray(hidden_states, np.float32)
    in_maps = _host_inputs(hs, np.asarray(latent), np.asarray(w_latent),
                           np.asarray(wq), np.asarray(wk), np.asarray(wv),
                           np.asarray(wo), np.asarray(w_latent_o))
    if "nc" not in _COMPILED:
        _COMPILED["nc"] = _build_kernel()
    nc = _COMPILED["nc"]
    res = run_bass_kernel_spmd(nc, in_maps, list(range(N_CORES)),
                               trace=_trace, trace_cores=_trace_cores,
                               tmpdir=_tmpdir)
    kernel.last_result = res
    out = np.empty((B, S, HID), np.float32)
    for c in range(N_CORES):
        b, g = c // 4, c % 4
        out[b, :, g * 512:(g + 1) * 512] = res.results[c]["y"]
    return out


kernel.last_result = None


# revision 22
# speedup vs baseline: 1.0948x; 1.0109x over previous
"""MLA (multi-head latent attention) Trainium2 kernel, 8-way tensor/data parallel.

Problem shapes (hardcoded): B=2, S=2048, HID=2048, H=16, KVH=4, DH=128, L=64.

Sharding: core c -> batch b = c//4, kv-group g = c%4.
Each core computes q-heads 4g..4g+3 and kv head g for its batch.

Stage A: q/k/v projections j-outer (4 seq-quarter PSUM accumulators per pass,
  so consecutive matmuls share the stationary weight chunk), RoPE on DVE with
  bf16 trig (q trig pre-scaled by 1/sqrt(DH) so attention exp needs no scale),
  j-major hsT DMA so the k pass streams behind the DMA; latent branch last.
Stage B: flat software pipeline over all 64 (head, sq, k2) steps:
  exp(t) | scores(t+1) | av(t-1) | pair-sum(t-1), with the softmax denominator
  reduced by a DVE/GPSIMD add tree to one [128,512] tile -> single ones-matmul,
  reciprocal on DVE (PSUM src, bf16 out), broadcast matmul in bf16, and the
  whole normalization of block b deferred into block b+1 so the PE never
  stalls.  AllGather per head; head 3 in two halves.
Stage C: o-projection consumes gathered heads in ARRIVAL order (j-outer),
  parking partial sums in SBUF f32; only head-3's matmuls trail the last
  AllGather; latent contribution (ReduceScattered earlier) folded in as a
  bf16 rank-1 update.
"""

import numpy as np
import ml_dtypes
from contextlib import ExitStack

import concourse.bass as bass
import concourse.bacc as bacc
import concourse.tile as tile
import concourse.mybir as mybir
from concourse.bass_utils import run_bass_kernel_spmd

BF16 = ml_dtypes.bfloat16
FP32 = mybir.dt.float32
BF16_DT = mybir.dt.bfloat16

B, S, HID = 2, 2048, 2048
H, KVH, DH, L = 16, 4, 128, 64
THETA = 10000.0
N_CORES = 8
GROUPS = [[0, 1, 2, 3], [4, 5, 6, 7]]
NJ = HID // 128          # 16 contraction chunks
NSQ = S // 512           # 4 seq blocks of 512
NSB = S // 128           # 16 seq blocks of 128
SCALE = 1.0 / float(np.sqrt(np.float32(DH)))
NBLK = 16                # 4 heads x 4 sq blocks
NT = NBLK * 8            # flat pipeline steps (one per k2 chunk-pair)

_COMPILED = {}


def _emit_body(nc, tc, ctx, d, single_core):
    EXP = mybir.ActivationFunctionType.Exp

    # ---------- persistent pools ----------
    qk_pool = ctx.enter_context(tc.tile_pool(name="qk", bufs=1))
    v_pool = ctx.enter_context(tc.tile_pool(name="v", bufs=1))
    attn_pool = ctx.enter_context(tc.tile_pool(name="attn", bufs=1))
    const_pool = ctx.enter_context(tc.tile_pool(name="const", bufs=1))
    dram_pool = ctx.enter_context(tc.tile_pool(name="dram", bufs=1, space="DRAM"))

    qkT = qk_pool.tile([128, 5 * S], BF16_DT)       # 4 roped+scaled q heads + k
    v_sb = v_pool.tile([128, S], BF16_DT)           # v in [s-part, dh] blocks
    attnT = attn_pool.tile([128, 4 * S], BF16_DT)   # normalized attnT

    ones_col = const_pool.tile([128, 1], BF16_DT)
    ones_row = const_pool.tile([1, 128], BF16_DT)
    ones_sq = const_pool.tile([128, 128], BF16_DT)
    ident = const_pool.tile([128, 128], BF16_DT)
    nc.vector.memset(ones_col[:], 1.0)
    nc.vector.memset(ones_row[:], 1.0)
    nc.vector.memset(ones_sq[:], 1.0)

    ag_in = [dram_pool.tile([128, S], BF16_DT, tag=f"agi{h}", name=f"ag_in{h}")
             for h in range(3)]
    ag_out = [dram_pool.tile([512, S], BF16_DT, tag=f"ago{h}", name=f"ag_out{h}")
              for h in range(3)]
    ag_in3 = [dram_pool.tile([128, S // 2], BF16_DT, tag=f"agi3{p}", name=f"ag_in3{p}")
              for p in range(2)]
    ag_out3 = [dram_pool.tile([512, S // 2], BF16_DT, tag=f"ago3{p}", name=f"ag_out3{p}")
               for p in range(2)]
    rs_in = dram_pool.tile([1, HID], BF16_DT)
    rs_out = dram_pool.tile([1, 512], BF16_DT)

    def run_ag(inp, outp):
        if single_core:
            nc.sync.dma_start(outp[0:128, :], inp[:])
        else:
            nc.gpsimd.collective_compute(
                "AllGather", mybir.AluOpType.bypass, replica_groups=GROUPS,
                ins=[inp.opt()], outs=[outp.opt()])

    # ---------- stage A: projections + rope (j-outer) ----------
    with ExitStack() as actx:
        hs_pool = actx.enter_context(tc.tile_pool(name="hs", bufs=1))
        w_pool = actx.enter_context(tc.tile_pool(name="w", bufs=1))
        trig_pool = actx.enter_context(tc.tile_pool(name="trig", bufs=1))
        rope_pool = actx.enter_context(tc.tile_pool(name="rope", bufs=3))

        wq_sb = w_pool.tile([128, NJ * 512], BF16_DT)
        wk_sb = w_pool.tile([128, NJ * 128], BF16_DT)
        wv_sb = w_pool.tile([128, NJ * 128], BF16_DT)
        cos_k = trig_pool.tile([128, S], BF16_DT)
        sin_k = trig_pool.tile([128, S], BF16_DT)
        cos_q = trig_pool.tile([128, S], BF16_DT)
        sin_q = trig_pool.tile([128, S], BF16_DT)
        hsT = hs_pool.tile([128, NJ * S], BF16_DT)

        nc.sync.dma_start(wk_sb[:], d["wk_sb"].ap())
        for j in range(NJ):
            nc.sync.dma_start(hsT[:, j * S:(j + 1) * S], d["hsT"][j])
        nc.sync.dma_start(wv_sb[:], d["wv_sb"].ap())
        nc.sync.dma_start(cos_k[:], d["cos_k"].ap())
        nc.sync.dma_start(sin_k[:], d["sin_k"].ap())
        nc.sync.dma_start(cos_q[:], d["cos_q"].ap())
        nc.sync.dma_start(sin_q[:], d["sin_q"].ap())
        nc.sync.dma_start(ident[:], d["ident"].ap())
        nc.sync.dma_start(wq_sb[:], d["wq_sb"].ap())

        lat0 = w_pool.tile([128, L], BF16_DT)
        wlat = w_pool.tile([128, 128], BF16_DT)
        wlo = w_pool.tile([128, HID], BF16_DT)
        nc.sync.dma_start(lat0[:], d["lat0T"].ap())
        nc.sync.dma_start(wlat[:], d["w_lat"].ap())
        nc.sync.dma_start(wlo[:], d["wlo"].ap())

        def project(pool, w_sb, w_off, w_stride):
            """j-outer projection pass -> 4 psum quarters [128,512]."""
            ps = [pool.tile([128, 512], FP32, tag="proj", name=f"ps{sq}")
                  for sq in range(NSQ)]
            for j in range(NJ):
                for sq in range(NSQ):
                    nc.tensor.matmul(
                        ps[sq][:],
                        lhsT=w_sb[:, w_off + j * w_stride:
                                  w_off + j * w_stride + 128],
                        rhs=hsT[:, j * S + sq * 512: j * S + sq * 512 + 512],
                        start=(j == 0), stop=(j == NJ - 1),
                    )
            return ps

        def rope(ps, dst, dst_off, cosT, sinT):
            for sq in range(NSQ):
                qf = rope_pool.tile([128, 512], FP32, tag="qf")
                nc.scalar.copy(qf[:], ps[sq][:])
                qs = rope_pool.tile([128, 512], FP32, tag="qs")
                nc.sync.dma_start(qs[0:64, :], qf[64:128, :])
                nc.sync.dma_start(qs[64:128, :], qf[0:64, :])
                m1 = rope_pool.tile([128, 512], FP32, tag="m1")
                nc.vector.tensor_mul(m1[:], qf[:], cosT[:, bass.ts(sq, 512)])
                m2 = rope_pool.tile([128, 512], FP32, tag="m2")
                nc.vector.tensor_mul(m2[:], qs[:], sinT[:, bass.ts(sq, 512)])
                nc.vector.tensor_add(
                    dst[:, dst_off + sq * 512: dst_off + sq * 512 + 512],
                    m1[:], m2[:])

        with ExitStack() as pctx:
            pA = pctx.enter_context(tc.tile_pool(name="pA", bufs=8, space="PSUM"))
            pT = pA
            # fused k+v pass: both stream behind the hsT chunk DMAs
            ps_k = [pA.tile([128, 512], FP32, tag="proj", name=f"psk{sq}")
                    for sq in range(NSQ)]
            ps_v = [pA.tile([128, 512], FP32, tag="proj", name=f"psv{sq}")
                    for sq in range(NSQ)]
            for j in range(NJ):
                for ps, w_sb in ((ps_k, wk_sb), (ps_v, wv_sb)):
                    for sq in range(NSQ):
                        nc.tensor.matmul(
                            ps[sq][:],
                            lhsT=w_sb[:, j * 128: j * 128 + 128],
                            rhs=hsT[:, j * S + sq * 512: j * S + sq * 512 + 512],
                            start=(j == 0), stop=(j == NJ - 1),
                        )
            rope(ps_k, qkT, 4 * S, cos_k, sin_k)
            for sq in range(NSQ):
                vT_bf = rope_pool.tile([128, 512], BF16_DT, tag="vbf")
                nc.scalar.copy(vT_bf[:], ps_v[sq][:])
                tp = pT.tile([128, 512], BF16_DT, tag="proj", name="tp")
                for i in range(4):
                    nc.tensor.transpose(
                        tp[:, bass.ts(i, 128)], vT_bf[:, bass.ts(i, 128)],
                        ident[:])
                nc.vector.tensor_copy(v_sb[:, bass.ts(sq, 512)], tp[:])

        # ---------- latent branch (tiny; RS overlaps q projections) ----------
        with ExitStack() as lctx:
            l_pool = lctx.enter_context(tc.tile_pool(name="lat", bufs=1))
            pLs = lctx.enter_context(tc.tile_pool(name="pLs", bufs=2, space="PSUM"))
            pLa = lctx.enter_context(tc.tile_pool(name="pLa", bufs=1, space="PSUM"))

            lp_ps = pLs.tile([128, L], FP32, tag="scL")
            nc.tensor.matmul(lp_ps[:], lhsT=wlat[:], rhs=lat0[:],
                             start=True, stop=True)
            latpT = l_pool.tile([128, L], BF16_DT)
            nc.scalar.copy(latpT[:], lp_ps[:])

            scL = pLs.tile([128, NSB * L], FP32, tag="scL")
            for kc in range(NSB):
                nc.tensor.matmul(
                    scL[:, bass.ts(kc, L)],
                    lhsT=qkT[:, 4 * S + kc * 128: 4 * S + kc * 128 + 128],
                    rhs=latpT[:], start=True, stop=True)
            exL = l_pool.tile([128, NSB * L], BF16_DT, tag="exL")
            nc.scalar.activation(exL[:], scL[:], EXP, scale=SCALE)
            avL = pLa.tile([128, L], FP32, tag="avL")
            denL = pLa.tile([1, L], FP32, tag="denL")
            for kc in range(NSB):
                nc.tensor.matmul(avL[:], lhsT=v_sb[:, bass.ts(kc, 128)],
                                 rhs=exL[:, bass.ts(kc, L)],
                                 start=(kc == 0), stop=(kc == NSB - 1))
                nc.tensor.matmul(denL[:], lhsT=ones_col[:],
                                 rhs=exL[:, bass.ts(kc, L)],
                                 start=(kc == 0), stop=(kc == NSB - 1))
            recipL = l_pool.tile([1, L], BF16_DT, tag="recipL")
            with nc.allow_low_precision(reason="softmax reciprocal in bf16"):
                nc.vector.reciprocal(recipL[:], denL[:])
            bcL = pLs.tile([128, L], FP32, tag="scL")
            nc.tensor.matmul(bcL[:], lhsT=ones_row[:], rhs=recipL[:],
                             start=True, stop=True)
            bcL_sb = l_pool.tile([128, L], FP32, tag="bcLsb")
            nc.vector.tensor_copy(bcL_sb[:], bcL[:])
            attLn = l_pool.tile([128, L], FP32, tag="attLn")
            nc.vector.tensor_mul(attLn[:], avL[:], bcL_sb[:])
            meanv = l_pool.tile([128, 1], FP32, tag="meanv")
            nc.vector.tensor_reduce(meanv[:], attLn[:],
                                    axis=mybir.AxisListType.X,
                                    op=mybir.AluOpType.add)
            nc.vector.tensor_scalar_mul(meanv[:], meanv[:], 1.0 / L)
            meanv_bf = l_pool.tile([128, 1], BF16_DT, tag="meanvbf")
            nc.vector.tensor_copy(meanv_bf[:], meanv[:])
            contrib_sb = l_pool.tile([1, HID], BF16_DT, tag="contrib")
            for i in range(4):
                cps = pLs.tile([1, 512], FP32, tag="scL")
                nc.tensor.matmul(cps[:], lhsT=meanv_bf[:],
                                 rhs=wlo[:, bass.ts(i, 512)],
                                 start=True, stop=True)
                with nc.allow_low_precision(reason="latent contribution bf16"):
                    nc.vector.tensor_copy(contrib_sb[:, bass.ts(i, 512)], cps[:])
            nc.sync.dma_start(rs_in[:], contrib_sb[:])

        if single_core:
            nc.sync.dma_start(rs_out[:], rs_in[:, 0:512])
        else:
            nc.gpsimd.collective_compute(
                "ReduceScatter", mybir.AluOpType.add, replica_groups=GROUPS,
                ins=[rs_in.opt()], outs=[rs_out.opt()])

        with ExitStack() as qctx:
            pA2 = qctx.enter_context(tc.tile_pool(name="pA2", bufs=6, space="PSUM"))
            for hh in range(4):
                rope(project(pA2, wq_sb, hh * 128, 512), qkT, hh * S,
                     cos_q, sin_q)

    # ---------- stage B: attention, flat software pipeline ----------
    with ExitStack() as bctx:
        e_pool = bctx.enter_context(tc.tile_pool(name="expd", bufs=4))
        s_pool = bctx.enter_context(tc.tile_pool(name="s1", bufs=4))
        t_pool = bctx.enter_context(tc.tile_pool(name="tree", bufs=3))
        n_pool = bctx.enter_context(tc.tile_pool(name="norm", bufs=2))
        pS = bctx.enter_context(tc.tile_pool(name="pS", bufs=2, space="PSUM"))
        pAV = bctx.enter_context(tc.tile_pool(name="pAV", bufs=2, space="PSUM"))
        pD = bctx.enter_context(tc.tile_pool(name="pD", bufs=2, space="PSUM"))

        # per-step state (indexed by flat step t = b*8 + k2)
        sc_t = {}    # t -> sc psum tile
        ex_t = {}    # t -> exp'd bf16 tile
        s1_b = {}    # b -> list of 8 pair-sum tiles
        rrow_b = {}  # b -> reciprocal row
        l2_b = {}    # b -> list of L2 tiles
        av_b = {}    # b -> av psum tile
        den_b = {}   # b -> den psum tile
        s4_b = {}    # b -> final tree tile

        def emit_sc(t):
            b, k2 = t // 8, t % 8
            hh, sq = b // 4, b % 4
            sc = pS.tile([128, 1024], FP32, tag="sc")
            for i in range(2):
                kc = k2 * 2 + i
                nc.tensor.matmul(
                    sc[:, bass.ts(i, 512)],
                    lhsT=qkT[:, 4 * S + kc * 128: 4 * S + kc * 128 + 128],
                    rhs=qkT[:, hh * S + sq * 512: hh * S + sq * 512 + 512],
                    start=True, stop=True)
            sc_t[t] = sc

        def emit_exp(t):
            ex = e_pool.tile([128, 1024], BF16_DT, tag="ex")
            nc.scalar.activation(ex[:], sc_t.pop(t)[:], EXP, scale=1.0)
            ex_t[t] = ex

        def emit_av_s1(t):
            b, k2 = t // 8, t % 8
            if k2 == 0:
                av_b[b] = pAV.tile([128, 512], FP32, tag="av", name=f"av{b}")
                s1_b[b] = []
            ex = ex_t.pop(t)
            for i in range(2):
                kc = k2 * 2 + i
                nc.tensor.matmul(
                    av_b[b][:], lhsT=v_sb[:, bass.ts(kc, 128)],
                    rhs=ex[:, bass.ts(i, 512)],
                    start=(kc == 0), stop=(kc == NSB - 1))
            s1 = s_pool.tile([128, 512], BF16_DT, tag="s1")
            nc.vector.tensor_add(s1[:], ex[:, 0:512], ex[:, 512:1024])
            s1_b[b].append(s1)
            # tree adds on gpsimd as pairs become ready
            if k2 in (1, 3, 5, 7):
                if k2 == 1:
                    l2_b[b] = []
                t2 = t_pool.tile([128, 512], BF16_DT, tag="l2")
                nc.gpsimd.tensor_add(t2[:], s1_b[b][k2 - 1][:], s1_b[b][k2][:])
                l2_b[b].append(t2)
            if k2 == 5:
                t3a = t_pool.tile([128, 512], BF16_DT, tag="l3")
                nc.gpsimd.tensor_add(t3a[:], l2_b[b][0][:], l2_b[b][1][:])
                l2_b[b].append(t3a)
            if k2 == 7:
                # l2_b now holds [L2_0, L2_1, L2_2, L3a, L2_3]; finish on DVE
                t3b = t_pool.tile([128, 512], BF16_DT, tag="l3")
                nc.vector.tensor_add(t3b[:], l2_b[b][2][:], l2_b[b][4][:])
                s4 = t_pool.tile([128, 512], BF16_DT, tag="l4")
                nc.vector.tensor_add(s4[:], l2_b[b][3][:], t3b[:])
                s4_b[b] = s4
                del s1_b[b], l2_b[b]

        def emit_den(b):
            den = pD.tile([1, 512], FP32, tag="db")
            nc.tensor.matmul(den[:], lhsT=ones_col[:], rhs=s4_b.pop(b)[:],
                             start=True, stop=True)
            den_b[b] = den

        def emit_recip(b):
            rrow = n_pool.tile([1, 512], BF16_DT, tag="rrow")
            with nc.allow_low_precision(reason="softmax reciprocal in bf16"):
                nc.vector.reciprocal(rrow[:], den_b.pop(b)[:])
            rrow_b[b] = rrow

        def emit_bcmul(b):
            hh, sq = b // 4, b % 4
            bc = pS.tile([128, 512], FP32, tag="sc")
            nc.tensor.matmul(bc[:], lhsT=ones_row[:], rhs=rrow_b.pop(b)[:],
                             start=True, stop=True)
            bc_sb = n_pool.tile([128, 512], FP32, tag="bcsb")
            nc.vector.tensor_copy(bc_sb[:], bc[:])
            nc.vector.tensor_mul(
                attnT[:, hh * S + sq * 512: hh * S + sq * 512 + 512],
                av_b.pop(b)[:], bc_sb[:])

        def ship(hh):
            if hh < 3:
                nc.sync.dma_start(ag_in[hh][:], attnT[:, hh * S:(hh + 1) * S])
                run_ag(ag_in[hh], ag_out[hh])
            elif hh == 3:
                nc.sync.dma_start(ag_in3[0][:], attnT[:, 3 * S: 3 * S + 1024])
                run_ag(ag_in3[0], ag_out3[0])
            else:  # second half of head 3
                nc.sync.dma_start(ag_in3[1][:],
                                  attnT[:, 3 * S + 1024: 3 * S + 2048])
                run_ag(ag_in3[1], ag_out3[1])

        emit_sc(0)
        for t in range(NT):
            b, k2 = t // 8, t % 8
            emit_exp(t)
            if t + 1 < NT:
                emit_sc(t + 1)
            if t >= 1:
                emit_av_s1(t - 1)
            if k2 == 2 and b >= 1:
                emit_den(b - 1)
            if k2 == 3 and b >= 1:
                emit_recip(b - 1)
            if k2 == 6 and b >= 1:
                emit_bcmul(b - 1)
            if k2 == 7 and b >= 1 and (b - 1) % 4 == 3 and b < 13:
                ship((b - 1) // 4)          # heads 0..2 complete & normed
            if k2 == 7 and b == 14:
                ship(3)                     # head-3 first half (sq 0,1 normed)
        emit_av_s1(NT - 1)
        emit_den(NBLK - 1)
        emit_recip(NBLK - 1)
        emit_bcmul(NBLK - 1)
        ship(4)                             # head-3 second half

    # ---------- stage C: output projection (arrival-ordered) ----------
    with ExitStack() as cctx:
        g_pool = cctx.enter_context(tc.tile_pool(name="gath", bufs=5))
        wo_pool = cctx.enter_context(tc.tile_pool(name="wo", bufs=1))
        oa_pool = cctx.enter_context(tc.tile_pool(name="oacc", bufs=1))
        o_pool = cctx.enter_context(tc.tile_pool(name="oev", bufs=3))
        pO = cctx.enter_context(tc.tile_pool(name="pO", bufs=3, space="PSUM"))

        wo_sb = wo_pool.tile([128, NJ * 512], BF16_DT)
        nc.sync.dma_start(wo_sb[:], d["wo_sb"].ap())
        latrow = wo_pool.tile([1, 512], BF16_DT)
        nc.sync.dma_start(latrow[:], rs_out[:])

        o_acc = oa_pool.tile([128, NSB * 512], FP32)

        # arrival order: heads 0..2 (whole), then head-3 half 0 / half 1.
        # All gather DMAs issue up front so they stream in as AGs land.
        parts = [(0, None), (1, None), (2, None), (3, 0), (3, 1)]
        garrs = {}
        for a, half in parts:
            if half is None:
                garr = g_pool.tile([128, 4 * S], BF16_DT, tag="garr",
                                   name=f"garr{a}")
                for r in range(4):
                    nc.sync.dma_start(garr[:, r * S:(r + 1) * S],
                                      ag_out[a][r * 128:(r + 1) * 128, :])
            else:
                garr = g_pool.tile([128, 4 * 1024], BF16_DT, tag="garr",
                                   name=f"garr3{half}")
                for r in range(4):
                    nc.sync.dma_start(
                        garr[:, r * 1024:(r + 1) * 1024],
                        ag_out3[half][r * 128:(r + 1) * 128, :])
            garrs[(a, half)] = garr
        for a, half in parts:
            garr = garrs[(a, half)]
            if half is None:
                sbs = range(NSB)
                cw = S
            else:
                sbs = range(half * 8, half * 8 + 8)
                cw = 1024
            for sb in sbs:
                sbo = sb - (half * 8 if half is not None else 0)
                ops = pO.tile([128, 512], FP32, tag="ops")
                for r in range(4):
                    j = 4 * r + a
                    nc.tensor.matmul(
                        ops[:],
                        lhsT=garr[:, r * cw + sbo * 128: r * cw + sbo * 128 + 128],
                        rhs=wo_sb[:, bass.ts(j, 512)],
                        start=(r == 0), stop=(r == 3 and a != 3))
                if a == 3:
                    nc.tensor.matmul(ops[:], lhsT=ones_row[:],
                                     rhs=latrow[:], start=False, stop=True)
                acc_sl = o_acc[:, sb * 512:(sb + 1) * 512]
                if a == 0:
                    nc.vector.tensor_copy(acc_sl, ops[:])
                elif a < 3:
                    nc.vector.tensor_add(acc_sl, acc_sl, ops[:])
                else:
                    oev = o_pool.tile([128, 512], FP32, tag="oev")
                    nc.vector.tensor_add(oev[:], acc_sl, ops[:])
                    nc.sync.dma_start(
                        d["y"].ap()[sb * 128:(sb + 1) * 128, :], oev[:])


def _build_kernel(reps=1, single_core=False):
    nc = bacc.Bacc("TRN2", target_bir_lowering=False, debug=False,
                   num_devices=(1 if single_core else N_CORES))

    d = {
        "hsT": nc.dram_tensor("hsT", [NJ, 128, S], BF16_DT, kind="ExternalInput"),
        "wq_sb": nc.dram_tensor("wq_sb", [128, NJ * 512], BF16_DT, kind="ExternalInput"),
        "wk_sb": nc.dram_tensor("wk_sb", [128, NJ * 128], BF16_DT, kind="ExternalInput"),
        "wv_sb": nc.dram_tensor("wv_sb", [128, NJ * 128], BF16_DT, kind="ExternalInput"),
        "wo_sb": nc.dram_tensor("wo_sb", [128, NJ * 512], BF16_DT, kind="ExternalInput"),
        "cos_k": nc.dram_tensor("cos_k", [128, S], BF16_DT, kind="ExternalInput"),
        "sin_k": nc.dram_tensor("sin_k", [128, S], BF16_DT, kind="ExternalInput"),
        "cos_q": nc.dram_tensor("cos_q", [128, S], BF16_DT, kind="ExternalInput"),
        "sin_q": nc.dram_tensor("sin_q", [128, S], BF16_DT, kind="ExternalInput"),
        "ident": nc.dram_tensor("ident", [128, 128], BF16_DT, kind="ExternalInput"),
        "lat0T": nc.dram_tensor("lat0T", [128, L], BF16_DT, kind="ExternalInput"),
        "w_lat": nc.dram_tensor("w_lat", [128, 128], BF16_DT, kind="ExternalInput"),
        "wlo": nc.dram_tensor("wlo", [128, HID], BF16_DT, kind="ExternalInput"),
        "y": nc.dram_tensor("y", [S, 512], FP32, kind="ExternalOutput"),
    }

    with tile.TileContext(nc) as tc:
        with ExitStack() as ctx:
            _emit_body(nc, tc, ctx, d, single_core)

    nc.compile()
    return nc


def _host_inputs(hs, latent, w_latent, wq, wk, wv, wo, wlo):
    """Build the 8 per-core input maps."""
    inv_freq = 1.0 / (THETA ** (np.arange(0, DH, 2, dtype=np.float32) / DH))
    t = np.arange(S, dtype=np.float32)
    freqs = np.outer(t, inv_freq)
    emb = np.concatenate([freqs, freqs], axis=-1)          # [S, DH]
    cosT = np.ascontiguousarray(np.cos(emb).T.astype(np.float32))
    sinT = np.sin(emb).T.astype(np.float32)
    sinS = sinT.copy()
    sinS[:64] *= -1.0
    cos_k = cosT.astype(BF16)
    sin_k = np.ascontiguousarray(sinS).astype(BF16)
    cos_q = (cosT * SCALE).astype(BF16)
    sin_q = (sinS * SCALE).astype(BF16)
    ident = np.eye(128, dtype=BF16)

    def chunked(w, cols):
        return np.ascontiguousarray(
            w.reshape(NJ, 128, cols).transpose(1, 0, 2).reshape(128, NJ * cols)
        ).astype(BF16)

    in_maps = []
    for c in range(N_CORES):
        b, g = c // 4, c % 4
        hsT = np.ascontiguousarray(hs[b].T).astype(BF16).reshape(NJ, 128, S)
        in_maps.append({
            "hsT": hsT,
            "wq_sb": chunked(wq[:, 4 * g * DH:(4 * g + 4) * DH], 512),
            "wk_sb": chunked(wk[:, g * DH:(g + 1) * DH], 128),
            "wv_sb": chunked(wv[:, g * DH:(g + 1) * DH], 128),
            "wo_sb": chunked(wo[:, g * 512:(g + 1) * 512], 512),
            "cos_k": cos_k, "sin_k": sin_k,
            "cos_q": cos_q, "sin_q": sin_q,
            "ident": ident,
            "lat0T": np.ascontiguousarray(latent[0, g].T).astype(BF16),
            "w_lat": w_latent.astype(BF16),
            "wlo": np.ascontiguousarray(wlo[g * DH:(g + 1) * DH, :]).astype(BF16),
        })
    return in_maps


def kernel(hidden_states, latent, w_latent, wq, wk, wv, wo, w_latent_o,
           _trace=False, _trace_cores=None, _tmpdir=None):
    hs = np.asarray(hidden_states, np.float32)
    in_maps = _host_inputs(hs, np.asarray(latent), np.asarray(w_latent),
                           np.asarray(wq), np.asarray(wk), np.asarray(wv),
                           np.asarray(wo), np.asarray(w_latent_o))
    if "nc" not in _COMPILED:
        _COMPILED["nc"] = _build_kernel()
    nc = _COMPILED["nc"]
    res = run_bass_kernel_spmd(nc, in_maps, list(range(N_CORES)),
                               trace=_trace, trace_cores=_trace_cores,
                               tmpdir=_tmpdir)
    kernel.last_result = res
    out = np.empty((B, S, HID), np.float32)
    for c in range(N_CORES):
        b, g = c // 4, c % 4
        out[b, :, g * 512:(g + 1) * 512] = res.results[c]["y"]
    return out


kernel.last_result = None


# revision 23
# speedup vs baseline: 1.1060x; 1.0102x over previous
"""MLA (multi-head latent attention) Trainium2 kernel, 8-way tensor/data parallel.

Problem shapes (hardcoded): B=2, S=2048, HID=2048, H=16, KVH=4, DH=128, L=64.

Sharding: core c -> batch b = c//4, kv-group g = c%4.
Each core computes q-heads 4g..4g+3 and kv head g for its batch.

Stage A: q/k/v projections j-outer (4 seq-quarter PSUM accumulators per pass,
  so consecutive matmuls share the stationary weight chunk), RoPE on DVE with
  bf16 trig (q trig pre-scaled by 1/sqrt(DH) so attention exp needs no scale),
  j-major hsT DMA so the k pass streams behind the DMA; latent branch last.
Stage B: flat software pipeline over all 64 (head, sq, k2) steps:
  exp(t) | scores(t+1) | av(t-1) | pair-sum(t-1), with the softmax denominator
  reduced by a DVE/GPSIMD add tree to one [128,512] tile -> single ones-matmul,
  reciprocal on DVE (PSUM src, bf16 out), broadcast matmul in bf16, and the
  whole normalization of block b deferred into block b+1 so the PE never
  stalls.  AllGather per head; head 3 in two halves.
Stage C: o-projection consumes gathered heads in ARRIVAL order (j-outer),
  parking partial sums in SBUF f32; only head-3's matmuls trail the last
  AllGather; latent contribution (ReduceScattered earlier) folded in as a
  bf16 rank-1 update.
"""

import numpy as np
import ml_dtypes
from contextlib import ExitStack

import concourse.bass as bass
import concourse.bacc as bacc
import concourse.tile as tile
import concourse.mybir as mybir
from concourse.bass_utils import run_bass_kernel_spmd

BF16 = ml_dtypes.bfloat16
FP32 = mybir.dt.float32
BF16_DT = mybir.dt.bfloat16

B, S, HID = 2, 2048, 2048
H, KVH, DH, L = 16, 4, 128, 64
THETA = 10000.0
N_CORES = 8
GROUPS = [[0, 1, 2, 3], [4, 5, 6, 7]]
NJ = HID // 128          # 16 contraction chunks
NSQ = S // 512           # 4 seq blocks of 512
NSB = S // 128           # 16 seq blocks of 128
SCALE = 1.0 / float(np.sqrt(np.float32(DH)))
NBLK = 16                # 4 heads x 4 sq blocks
NT = NBLK * 8            # flat pipeline steps (one per k2 chunk-pair)

_COMPILED = {}


def _emit_body(nc, tc, ctx, d, single_core):
    EXP = mybir.ActivationFunctionType.Exp

    # ---------- persistent pools ----------
    qk_pool = ctx.enter_context(tc.tile_pool(name="qk", bufs=1))
    v_pool = ctx.enter_context(tc.tile_pool(name="v", bufs=1))
    attn_pool = ctx.enter_context(tc.tile_pool(name="attn", bufs=1))
    const_pool = ctx.enter_context(tc.tile_pool(name="const", bufs=1))
    dram_pool = ctx.enter_context(tc.tile_pool(name="dram", bufs=1, space="DRAM"))

    qkT = qk_pool.tile([128, 5 * S], BF16_DT)       # 4 roped+scaled q heads + k
    v_sb = v_pool.tile([128, S], BF16_DT)           # v in [s-part, dh] blocks
    attnT = attn_pool.tile([128, 4 * S], BF16_DT)   # normalized attnT

    ones_col = const_pool.tile([128, 1], BF16_DT)
    ones_row = const_pool.tile([1, 128], BF16_DT)
    ones_sq = const_pool.tile([128, 128], BF16_DT)
    ident = const_pool.tile([128, 128], BF16_DT)
    nc.vector.memset(ones_col[:], 1.0)
    nc.vector.memset(ones_row[:], 1.0)
    nc.vector.memset(ones_sq[:], 1.0)

    ag_in = [dram_pool.tile([128, S], BF16_DT, tag=f"agi{h}", name=f"ag_in{h}")
             for h in range(3)]
    ag_out = [dram_pool.tile([512, S], BF16_DT, tag=f"ago{h}", name=f"ag_out{h}")
              for h in range(3)]
    ag_in3 = [dram_pool.tile([128, S // 2], BF16_DT, tag=f"agi3{p}", name=f"ag_in3{p}")
              for p in range(2)]
    ag_out3 = [dram_pool.tile([512, S // 2], BF16_DT, tag=f"ago3{p}", name=f"ag_out3{p}")
               for p in range(2)]
    rs_in = dram_pool.tile([1, HID], BF16_DT)
    rs_out = dram_pool.tile([1, 512], BF16_DT)

    def run_ag(inp, outp):
        if single_core:
            nc.sync.dma_start(outp[0:128, :], inp[:])
        else:
            nc.gpsimd.collective_compute(
                "AllGather", mybir.AluOpType.bypass, replica_groups=GROUPS,
                ins=[inp.opt()], outs=[outp.opt()])

    # ---------- stage A: projections + rope (j-outer) ----------
    with ExitStack() as actx:
        hs_pool = actx.enter_context(tc.tile_pool(name="hs", bufs=1))
        w_pool = actx.enter_context(tc.tile_pool(name="w", bufs=1))
        trig_pool = actx.enter_context(tc.tile_pool(name="trig", bufs=1))
        rope_pool = actx.enter_context(tc.tile_pool(name="rope", bufs=3))

        wq_sb = w_pool.tile([128, NJ * 512], BF16_DT)
        wk_sb = w_pool.tile([128, NJ * 128], BF16_DT)
        wv_sb = w_pool.tile([128, NJ * 128], BF16_DT)
        cos_k = trig_pool.tile([128, S], BF16_DT)
        sin_k = trig_pool.tile([128, S], BF16_DT)
        cos_q = trig_pool.tile([128, S], BF16_DT)
        sin_q = trig_pool.tile([128, S], BF16_DT)
        hsT = hs_pool.tile([128, NJ * S], BF16_DT)

        nc.sync.dma_start(wk_sb[:], d["wk_sb"].ap())
        for j in range(NJ):
            nc.sync.dma_start(hsT[:, j * S:(j + 1) * S], d["hsT"][j])
        nc.sync.dma_start(wv_sb[:], d["wv_sb"].ap())
        nc.sync.dma_start(cos_k[:], d["cos_k"].ap())
        nc.sync.dma_start(sin_k[:], d["sin_k"].ap())
        nc.sync.dma_start(cos_q[:], d["cos_q"].ap())
        nc.sync.dma_start(sin_q[:], d["sin_q"].ap())
        nc.sync.dma_start(ident[:], d["ident"].ap())
        nc.sync.dma_start(wq_sb[:], d["wq_sb"].ap())

        lat0 = w_pool.tile([128, L], BF16_DT)
        wlat = w_pool.tile([128, 128], BF16_DT)
        wlo = w_pool.tile([128, HID], BF16_DT)
        nc.sync.dma_start(lat0[:], d["lat0T"].ap())
        nc.sync.dma_start(wlat[:], d["w_lat"].ap())
        nc.sync.dma_start(wlo[:], d["wlo"].ap())

        def project(pool, w_sb, w_off, w_stride):
            """j-outer projection pass -> 4 psum quarters [128,512]."""
            ps = [pool.tile([128, 512], FP32, tag="proj", name=f"ps{sq}")
                  for sq in range(NSQ)]
            for j in range(NJ):
                for sq in range(NSQ):
                    nc.tensor.matmul(
                        ps[sq][:],
                        lhsT=w_sb[:, w_off + j * w_stride:
                                  w_off + j * w_stride + 128],
                        rhs=hsT[:, j * S + sq * 512: j * S + sq * 512 + 512],
                        start=(j == 0), stop=(j == NJ - 1),
                    )
            return ps

        def rope(ps, dst, dst_off, cosT, sinT):
            for sq in range(NSQ):
                qf = rope_pool.tile([128, 512], FP32, tag="qf")
                nc.scalar.copy(qf[:], ps[sq][:])
                qs = rope_pool.tile([128, 512], FP32, tag="qs")
                nc.sync.dma_start(qs[0:64, :], qf[64:128, :])
                nc.sync.dma_start(qs[64:128, :], qf[0:64, :])
                m1 = rope_pool.tile([128, 512], FP32, tag="m1")
                nc.vector.tensor_mul(m1[:], qf[:], cosT[:, bass.ts(sq, 512)])
                m2 = rope_pool.tile([128, 512], FP32, tag="m2")
                nc.vector.tensor_mul(m2[:], qs[:], sinT[:, bass.ts(sq, 512)])
                nc.vector.tensor_add(
                    dst[:, dst_off + sq * 512: dst_off + sq * 512 + 512],
                    m1[:], m2[:])

        with ExitStack() as pctx:
            pA = pctx.enter_context(tc.tile_pool(name="pA", bufs=8, space="PSUM"))
            pT = pA
            # fused k+v pass: both stream behind the hsT chunk DMAs
            ps_k = [pA.tile([128, 512], FP32, tag="proj", name=f"psk{sq}")
                    for sq in range(NSQ)]
            ps_v = [pA.tile([128, 512], FP32, tag="proj", name=f"psv{sq}")
                    for sq in range(NSQ)]
            for j in range(NJ):
                for ps, w_sb in ((ps_k, wk_sb), (ps_v, wv_sb)):
                    for sq in range(NSQ):
                        nc.tensor.matmul(
                            ps[sq][:],
                            lhsT=w_sb[:, j * 128: j * 128 + 128],
                            rhs=hsT[:, j * S + sq * 512: j * S + sq * 512 + 512],
                            start=(j == 0), stop=(j == NJ - 1),
                        )
            rope(ps_k, qkT, 4 * S, cos_k, sin_k)
            for sq in range(NSQ):
                vT_bf = rope_pool.tile([128, 512], BF16_DT, tag="vbf")
                nc.scalar.copy(vT_bf[:], ps_v[sq][:])
                tp = pT.tile([128, 512], BF16_DT, tag="proj", name="tp")
                for i in range(4):
                    nc.tensor.transpose(
                        tp[:, bass.ts(i, 128)], vT_bf[:, bass.ts(i, 128)],
                        ident[:])
                nc.vector.tensor_copy(v_sb[:, bass.ts(sq, 512)], tp[:])

        # ---------- latent branch (tiny; RS overlaps q projections) ----------
        with ExitStack() as lctx:
            l_pool = lctx.enter_context(tc.tile_pool(name="lat", bufs=1))
            pLs = lctx.enter_context(tc.tile_pool(name="pLs", bufs=2, space="PSUM"))
            pLa = lctx.enter_context(tc.tile_pool(name="pLa", bufs=1, space="PSUM"))

            lp_ps = pLs.tile([128, L], FP32, tag="scL")
            nc.tensor.matmul(lp_ps[:], lhsT=wlat[:], rhs=lat0[:],
                             start=True, stop=True)
            latpT = l_pool.tile([128, L], BF16_DT)
            nc.scalar.copy(latpT[:], lp_ps[:])

            scL = pLs.tile([128, NSB * L], FP32, tag="scL")
            for kc in range(NSB):
                nc.tensor.matmul(
                    scL[:, bass.ts(kc, L)],
                    lhsT=qkT[:, 4 * S + kc * 128: 4 * S + kc * 128 + 128],
                    rhs=latpT[:], start=True, stop=True)
            exL = l_pool.tile([128, NSB * L], BF16_DT, tag="exL")
            nc.scalar.activation(exL[:], scL[:], EXP, scale=SCALE)
            avL = pLa.tile([128, L], FP32, tag="avL")
            denL = pLa.tile([1, L], FP32, tag="denL")
            for kc in range(NSB):
                nc.tensor.matmul(avL[:], lhsT=v_sb[:, bass.ts(kc, 128)],
                                 rhs=exL[:, bass.ts(kc, L)],
                                 start=(kc == 0), stop=(kc == NSB - 1))
                nc.tensor.matmul(denL[:], lhsT=ones_col[:],
                                 rhs=exL[:, bass.ts(kc, L)],
                                 start=(kc == 0), stop=(kc == NSB - 1))
            recipL = l_pool.tile([1, L], BF16_DT, tag="recipL")
            with nc.allow_low_precision(reason="softmax reciprocal in bf16"):
                nc.vector.reciprocal(recipL[:], denL[:])
            bcL = pLs.tile([128, L], FP32, tag="scL")
            nc.tensor.matmul(bcL[:], lhsT=ones_row[:], rhs=recipL[:],
                             start=True, stop=True)
            bcL_sb = l_pool.tile([128, L], FP32, tag="bcLsb")
            nc.vector.tensor_copy(bcL_sb[:], bcL[:])
            attLn = l_pool.tile([128, L], FP32, tag="attLn")
            nc.vector.tensor_mul(attLn[:], avL[:], bcL_sb[:])
            meanv = l_pool.tile([128, 1], FP32, tag="meanv")
            nc.vector.tensor_reduce(meanv[:], attLn[:],
                                    axis=mybir.AxisListType.X,
                                    op=mybir.AluOpType.add)
            nc.vector.tensor_scalar_mul(meanv[:], meanv[:], 1.0 / L)
            meanv_bf = l_pool.tile([128, 1], BF16_DT, tag="meanvbf")
            nc.vector.tensor_copy(meanv_bf[:], meanv[:])
            contrib_sb = l_pool.tile([1, HID], BF16_DT, tag="contrib")
            for i in range(4):
                cps = pLs.tile([1, 512], FP32, tag="scL")
                nc.tensor.matmul(cps[:], lhsT=meanv_bf[:],
                                 rhs=wlo[:, bass.ts(i, 512)],
                                 start=True, stop=True)
                with nc.allow_low_precision(reason="latent contribution bf16"):
                    nc.vector.tensor_copy(contrib_sb[:, bass.ts(i, 512)], cps[:])
            nc.sync.dma_start(rs_in[:], contrib_sb[:])

        if single_core:
            nc.sync.dma_start(rs_out[:], rs_in[:, 0:512])
        else:
            nc.gpsimd.collective_compute(
                "ReduceScatter", mybir.AluOpType.add, replica_groups=GROUPS,
                ins=[rs_in.opt()], outs=[rs_out.opt()])

        with ExitStack() as qctx:
            pA2 = qctx.enter_context(tc.tile_pool(name="pA2", bufs=6, space="PSUM"))
            for hh in range(4):
                rope(project(pA2, wq_sb, hh * 128, 512), qkT, hh * S,
                     cos_q, sin_q)

    # ---------- stage B: attention, flat software pipeline ----------
    with ExitStack() as bctx:
        e_pool = bctx.enter_context(tc.tile_pool(name="expd", bufs=4))
        s_pool = bctx.enter_context(tc.tile_pool(name="s1", bufs=4))
        t_pool = bctx.enter_context(tc.tile_pool(name="tree", bufs=3))
        n_pool = bctx.enter_context(tc.tile_pool(name="norm", bufs=2))
        pS = bctx.enter_context(tc.tile_pool(name="pS", bufs=2, space="PSUM"))
        pAV = bctx.enter_context(tc.tile_pool(name="pAV", bufs=2, space="PSUM"))
        pD = bctx.enter_context(tc.tile_pool(name="pD", bufs=2, space="PSUM"))

        # per-step state (indexed by flat step t = b*8 + k2)
        sc_t = {}    # t -> sc psum tile
        ex_t = {}    # t -> exp'd bf16 tile
        s1_b = {}    # b -> list of 8 pair-sum tiles
        rrow_b = {}  # b -> reciprocal row
        l2_b = {}    # b -> list of L2 tiles
        av_b = {}    # b -> av psum tile
        den_b = {}   # b -> den psum tile
        s4_b = {}    # b -> final tree tile

        def emit_sc(t):
            b, k2 = t // 8, t % 8
            hh, sq = b // 4, b % 4
            sc = pS.tile([128, 1024], FP32, tag="sc")
            for i in range(2):
                kc = k2 * 2 + i
                nc.tensor.matmul(
                    sc[:, bass.ts(i, 512)],
                    lhsT=qkT[:, 4 * S + kc * 128: 4 * S + kc * 128 + 128],
                    rhs=qkT[:, hh * S + sq * 512: hh * S + sq * 512 + 512],
                    start=True, stop=True)
            sc_t[t] = sc

        def emit_exp(t):
            ex = e_pool.tile([128, 1024], BF16_DT, tag="ex")
            nc.scalar.activation(ex[:], sc_t.pop(t)[:], EXP, scale=1.0)
            ex_t[t] = ex

        def emit_av_s1(t):
            b, k2 = t // 8, t % 8
            last = (b == NBLK - 1)
            if k2 == 0:
                av_b[b] = pAV.tile([128, 512], FP32, tag="av", name=f"av{b}")
                s1_b[b] = []
                if last:
                    den_b[b] = pD.tile([1, 512], FP32, tag="db", name="den15")
            ex = ex_t.pop(t)
            for i in range(2):
                kc = k2 * 2 + i
                nc.tensor.matmul(
                    av_b[b][:], lhsT=v_sb[:, bass.ts(kc, 128)],
                    rhs=ex[:, bass.ts(i, 512)],
                    start=(kc == 0), stop=(kc == NSB - 1))
            s1 = s_pool.tile([128, 512], BF16_DT, tag="s1")
            nc.vector.tensor_add(s1[:], ex[:, 0:512], ex[:, 512:1024])
            s1_b[b].append(s1)
            if last:
                # final block: accumulate den directly on the PE so the
                # epilogue doesn't wait for the gpsimd tree
                nc.tensor.matmul(den_b[b][:], lhsT=ones_col[:], rhs=s1[:],
                                 start=(k2 == 0), stop=(k2 == 7))
                return
            # tree adds on gpsimd as pairs become ready
            if k2 in (1, 3, 5, 7):
                if k2 == 1:
                    l2_b[b] = []
                t2 = t_pool.tile([128, 512], BF16_DT, tag="l2")
                nc.gpsimd.tensor_add(t2[:], s1_b[b][k2 - 1][:], s1_b[b][k2][:])
                l2_b[b].append(t2)
            if k2 == 5:
                t3a = t_pool.tile([128, 512], BF16_DT, tag="l3")
                nc.gpsimd.tensor_add(t3a[:], l2_b[b][0][:], l2_b[b][1][:])
                l2_b[b].append(t3a)
            if k2 == 7:
                # l2_b now holds [L2_0, L2_1, L2_2, L3a, L2_3]; finish on DVE
                t3b = t_pool.tile([128, 512], BF16_DT, tag="l3")
                nc.vector.tensor_add(t3b[:], l2_b[b][2][:], l2_b[b][4][:])
                s4 = t_pool.tile([128, 512], BF16_DT, tag="l4")
                nc.vector.tensor_add(s4[:], l2_b[b][3][:], t3b[:])
                s4_b[b] = s4
                del s1_b[b], l2_b[b]

        def emit_den(b):
            den = pD.tile([1, 512], FP32, tag="db")
            nc.tensor.matmul(den[:], lhsT=ones_col[:], rhs=s4_b.pop(b)[:],
                             start=True, stop=True)
            den_b[b] = den

        def emit_recip(b):
            rrow = n_pool.tile([1, 512], BF16_DT, tag="rrow")
            with nc.allow_low_precision(reason="softmax reciprocal in bf16"):
                nc.vector.reciprocal(rrow[:], den_b.pop(b)[:])
            rrow_b[b] = rrow

        def emit_bcmul(b):
            hh, sq = b // 4, b % 4
            bc = pS.tile([128, 512], FP32, tag="sc")
            nc.tensor.matmul(bc[:], lhsT=ones_row[:], rhs=rrow_b.pop(b)[:],
                             start=True, stop=True)
            bc_sb = n_pool.tile([128, 512], FP32, tag="bcsb")
            nc.vector.tensor_copy(bc_sb[:], bc[:])
            nc.vector.tensor_mul(
                attnT[:, hh * S + sq * 512: hh * S + sq * 512 + 512],
                av_b.pop(b)[:], bc_sb[:])

        def ship(hh):
            if hh < 3:
                nc.sync.dma_start(ag_in[hh][:], attnT[:, hh * S:(hh + 1) * S])
                run_ag(ag_in[hh], ag_out[hh])
            elif hh == 3:
                nc.sync.dma_start(ag_in3[0][:], attnT[:, 3 * S: 3 * S + 1024])
                run_ag(ag_in3[0], ag_out3[0])
            else:  # second half of head 3
                nc.sync.dma_start(ag_in3[1][:],
                                  attnT[:, 3 * S + 1024: 3 * S + 2048])
                run_ag(ag_in3[1], ag_out3[1])

        emit_sc(0)
        for t in range(NT):
            b, k2 = t // 8, t % 8
            emit_exp(t)
            if t + 1 < NT:
                emit_sc(t + 1)
            if t >= 1:
                emit_av_s1(t - 1)
            if k2 == 2 and b >= 1:
                emit_den(b - 1)
            if k2 == 3 and b >= 1:
                emit_recip(b - 1)
            if k2 == 6 and b >= 1:
                emit_bcmul(b - 1)
            if k2 == 7 and b >= 1 and (b - 1) % 4 == 3 and b < 13:
                ship((b - 1) // 4)          # heads 0..2 complete & normed
            if k2 == 7 and b == 14:
                ship(3)                     # head-3 first half (sq 0,1 normed)
        emit_av_s1(NT - 1)
        emit_recip(NBLK - 1)
        emit_bcmul(NBLK - 1)
        ship(4)                             # head-3 second half

    # ---------- stage C: output projection (arrival-ordered) ----------
    with ExitStack() as cctx:
        g_pool = cctx.enter_context(tc.tile_pool(name="gath", bufs=3))
        wo_pool = cctx.enter_context(tc.tile_pool(name="wo", bufs=1))
        oa_pool = cctx.enter_context(tc.tile_pool(name="oacc", bufs=1))
        o_pool = cctx.enter_context(tc.tile_pool(name="oev", bufs=3))
        pO = cctx.enter_context(tc.tile_pool(name="pO", bufs=3, space="PSUM"))

        wo_sb = wo_pool.tile([128, NJ * 512], BF16_DT)
        nc.sync.dma_start(wo_sb[:], d["wo_sb"].ap())
        latrow = wo_pool.tile([1, 512], BF16_DT)
        nc.sync.dma_start(latrow[:], rs_out[:])

        o_acc = oa_pool.tile([128, NSB * 512], FP32)

        # arrival order: heads 0..2 (whole), then head-3 half 0 / half 1.
        # All gather DMAs issue up front so they stream in as AGs land.
        parts = [(0, None), (1, None), (2, None), (3, 0), (3, 1)]
        garrs = {}
        for a, half in parts:
            if half is None:
                garr = g_pool.tile([128, 4 * S], BF16_DT, tag="garr",
                                   name=f"garr{a}")
                for r in range(4):
                    nc.sync.dma_start(garr[:, r * S:(r + 1) * S],
                                      ag_out[a][r * 128:(r + 1) * 128, :])
            else:
                garr = g_pool.tile([128, 4 * 1024], BF16_DT, tag="garr",
                                   name=f"garr3{half}")
                for r in range(4):
                    nc.sync.dma_start(
                        garr[:, r * 1024:(r + 1) * 1024],
                        ag_out3[half][r * 128:(r + 1) * 128, :])
            garrs[(a, half)] = garr
        for a, half in parts:
            garr = garrs[(a, half)]
            if half is None:
                sbs = range(NSB)
                cw = S
            else:
                sbs = range(half * 8, half * 8 + 8)
                cw = 1024
            for sb in sbs:
                sbo = sb - (half * 8 if half is not None else 0)
                ops = pO.tile([128, 512], FP32, tag="ops")
                for r in range(4):
                    j = 4 * r + a
                    nc.tensor.matmul(
                        ops[:],
                        lhsT=garr[:, r * cw + sbo * 128: r * cw + sbo * 128 + 128],
                        rhs=wo_sb[:, bass.ts(j, 512)],
                        start=(r == 0), stop=(r == 3 and a != 3))
                if a == 3:
                    nc.tensor.matmul(ops[:], lhsT=ones_row[:],
                                     rhs=latrow[:], start=False, stop=True)
                acc_sl = o_acc[:, sb * 512:(sb + 1) * 512]
                if a == 0:
                    nc.vector.tensor_copy(acc_sl, ops[:])
                elif a < 3:
                    nc.vector.tensor_add(acc_sl, acc_sl, ops[:])
                else:
                    oev = o_pool.tile([128, 512], FP32, tag="oev")
                    nc.vector.tensor_add(oev[:], acc_sl, ops[:])
                    nc.sync.dma_start(
                        d["y"].ap()[sb * 128:(sb + 1) * 128, :], oev[:])


def _build_kernel(reps=1, single_core=False):
    nc = bacc.Bacc("TRN2", target_bir_lowering=False, debug=False,
                   num_devices=(1 if single_core else N_CORES))

    d = {
        "hsT": nc.dram_tensor("hsT", [NJ, 128, S], BF16_DT, kind="ExternalInput"),
        "wq_sb": nc.dram_tensor("wq_sb", [128, NJ * 512], BF16_DT, kind="ExternalInput"),
        "wk_sb": nc.dram_tensor("wk_sb", [128, NJ * 128], BF16_DT, kind="ExternalInput"),
        "wv_sb": nc.dram_tensor("wv_sb", [128, NJ * 128], BF16_DT, kind="ExternalInput"),
        "wo_sb": nc.dram_tensor("wo_sb", [128, NJ * 512], BF16_DT, kind="ExternalInput"),
        "cos_k": nc.dram_tensor("cos_k", [128, S], BF16_DT, kind="ExternalInput"),
        "sin_k": nc.dram_tensor("sin_k", [128, S], BF16_DT, kind="ExternalInput"),
        "cos_q": nc.dram_tensor("cos_q", [128, S], BF16_DT, kind="ExternalInput"),
        "sin_q": nc.dram_tensor("sin_q", [128, S], BF16_DT, kind="ExternalInput"),
        "ident": nc.dram_tensor("ident", [128, 128], BF16_DT, kind="ExternalInput"),
        "lat0T": nc.dram_tensor("lat0T", [128, L], BF16_DT, kind="ExternalInput"),
        "w_lat": nc.dram_tensor("w_lat", [128, 128], BF16_DT, kind="ExternalInput"),
        "wlo": nc.dram_tensor("wlo", [128, HID], BF16_DT, kind="ExternalInput"),
        "y": nc.dram_tensor("y", [S, 512], FP32, kind="ExternalOutput"),
    }

    with tile.TileContext(nc) as tc:
        with ExitStack() as ctx:
            _emit_body(nc, tc, ctx, d, single_core)

    nc.compile()
    return nc


def _host_inputs(hs, latent, w_latent, wq, wk, wv, wo, wlo):
    """Build the 8 per-core input maps."""
    inv_freq = 1.0 / (THETA ** (np.arange(0, DH, 2, dtype=np.float32) / DH))
    t = np.arange(S, dtype=np.float32)
    freqs = np.outer(t, inv_freq)
    emb = np.concatenate([freqs, freqs], axis=-1)          # [S, DH]
    cosT = np.ascontiguousarray(np.cos(emb).T.astype(np.float32))
    sinT = np.sin(emb).T.astype(np.float32)
    sinS = sinT.copy()
    sinS[:64] *= -1.0
    cos_k = cosT.astype(BF16)
    sin_k = np.ascontiguousarray(sinS).astype(BF16)
    cos_q = (cosT * SCALE).astype(BF16)
    sin_q = (sinS * SCALE).astype(BF16)
    ident = np.eye(128, dtype=BF16)

    def chunked(w, cols):
        return np.ascontiguousarray(
            w.reshape(NJ, 128, cols).transpose(1, 0, 2).reshape(128, NJ * cols)
        ).astype(BF16)

    in_maps = []
    for c in range(N_CORES):
        b, g = c // 4, c % 4
        hsT = np.ascontiguousarray(hs[b].T).astype(BF16).reshape(NJ, 128, S)
        in_maps.append({
            "hsT": hsT,
            "wq_sb": chunked(wq[:, 4 * g * DH:(4 * g + 4) * DH], 512),
            "wk_sb": chunked(wk[:, g * DH:(g + 1) * DH], 128),
            "wv_sb": chunked(wv[:, g * DH:(g + 1) * DH], 128),
            "wo_sb": chunked(wo[:, g * 512:(g + 1) * 512], 512),
            "cos_k": cos_k, "sin_k": sin_k,
            "cos_q": cos_q, "sin_q": sin_q,
            "ident": ident,
            "lat0T": np.ascontiguousarray(latent[0, g].T).astype(BF16),
            "w_lat": w_latent.astype(BF16),
            "wlo": np.ascontiguousarray(wlo[g * DH:(g + 1) * DH, :]).astype(BF16),
        })
    return in_maps


def kernel(hidden_states, latent, w_latent, wq, wk, wv, wo, w_latent_o,
           _trace=False, _trace_cores=None, _tmpdir=None):
    hs = np.asarray(hidden_states, np.float32)
    in_maps = _host_inputs(hs, np.asarray(latent), np.asarray(w_latent),
                           np.asarray(wq), np.asarray(wk), np.asarray(wv),
                           np.asarray(wo), np.asarray(w_latent_o))
    if "nc" not in _COMPILED:
        _COMPILED["nc"] = _build_kernel()
    nc = _COMPILED["nc"]
    res = run_bass_kernel_spmd(nc, in_maps, list(range(N_CORES)),
                               trace=_trace, trace_cores=_trace_cores,
                               tmpdir=_tmpdir)
    kernel.last_result = res
    out = np.empty((B, S, HID), np.float32)
    for c in range(N_CORES):
        b, g = c // 4, c % 4
        out[b, :, g * 512:(g + 1) * 512] = res.results[c]["y"]
    return out


kernel.last_result = None
